# revision 7
# baseline (speedup 1.0000x reference)
"""Fastmax (p=1 causal linear attention) Trainium2 kernel, 8-core SPMD, v5.

Sharding: data-parallel over heads (16 heads -> 2 per core). Each core
computes q/k/v projections for its 2 heads, chunked causal linear attention
(augmented [65,65] prefix state per head), and a partial output projection;
the host sums the 8 partial outputs and adds the bias.

v5 schedule/copy-count rewrite:
  - One shared [128,512]-f32 PSUM ring (4 bufs) serves projections, norm
    rows, scores, attention o tiles and out-proj tiles; transposes run in a
    single [128,512]-bf16 4-slot tile in chunk PAIRS whose results leave in
    ONE strided copy per pair (half the transpose-evacuation ops).
  - Stage A: q/k projections + per-token norms + k transposes (PE-bound);
    the norm finalize runs on DVE/Act under the v projections; q is scaled
    once by s; scores then need no further scaling and a0 is folded into
    mf = (ptj + 1) * [tri|ones|tri] (one fused DVE op per head per span).
  - Stage B interleaves v transposes (two chunks ahead), the prefix-state
    chain, scores, attention, and the one-span-delayed output projection.
  - State snapshots are ONE [65,65] copy per head (base partition 0); the
    head-1 q rows are replicated to partitions 0:63 once (qh2b) so the
    state matmuls' operands share base partitions; the prefix ones-row term
    uses a ones lhsT at partition 64 matching the snapshot row.
"""

import sys

sys.path.insert(0, "/opt/trn_rl_repo")

import numpy as np

B, N, D_MODEL, H, D_HEAD = 1, 2048, 1024, 16, 64
NCORES = 8
HPC = H // NCORES  # heads per core
DPC = HPC * D_HEAD  # out dims per core (128)
CH = 128  # chunk (tokens)
SPAN = 256  # query span (2 chunks)
NSPAN = N // SPAN
NCH = N // CH
KT = D_MODEL // 128  # contraction tiles for projections
NT = N // 512  # 512-wide column chunks of the sequence
RST = 136  # row-buffer stride per chunk (2*(64 data + ones col) + pad)

_CACHE = {}
DBG = None
MARKS = []


def _build():
    import concourse.bass as bass
    import concourse.tile as tile
    import concourse.mybir as mybir
    from concourse import bacc
    from concourse.alu_op_type import AluOpType

    BF = mybir.dt.bfloat16
    F32 = mybir.dt.float32
    AF = mybir.ActivationFunctionType
    AX = mybir.AxisListType

    nc = bacc.Bacc("TRN2", target_bir_lowering=False, debug=False, num_devices=NCORES)

    # xp: host-packed X, col n0*4096 + k*512 + c <-> X[tok n0*512+c, dm k*128+p]
    xp_d = nc.declare_dram_parameter("xp", [128, NT * KT * 512], BF, isOutput=False)
    wq_d = nc.declare_dram_parameter("wq", [128, D_MODEL], BF, isOutput=False)
    wk_d = nc.declare_dram_parameter("wk", [128, D_MODEL], BF, isOutput=False)
    wv_d = nc.declare_dram_parameter("wv", [128, D_MODEL], BF, isOutput=False)
    wo_d = nc.declare_dram_parameter("wo", [DPC, D_MODEL], BF, isOutput=False)
    consts_d = nc.declare_dram_parameter("consts", [128, 644], BF, isOutput=False)
    out_d = nc.declare_dram_parameter("out", [N, D_MODEL], BF, isOutput=True)
    dbg_d = nc.declare_dram_parameter("dbg", [128, 4352], BF, isOutput=True) if DBG else None

    # engine load balancer for PSUM->SBUF copies & small ops (cost-model based)
    load = {"v": 0.0, "s": 0.0}

    def _cost(eng, n, psum_src=True, mult=1.0):
        if eng == "v":
            return n * 1.04 * mult + (125.0 if psum_src else 60.0)
        return n * 0.92 + (160.0 if psum_src else 200.0)

    def mark(label):
        MARKS.append((label, int(nc.get_next_instruction_name()[2:])))

    with tile.TileContext(nc) as tc:

        def pick(n, psum_src=True, mult=1.0):
            cv = load["v"] + _cost("v", n, psum_src, mult)
            cs = load["s"] + _cost("s", n, psum_src)
            if cv <= cs:
                load["v"] = cv
                return "v"
            load["s"] = cs
            return "s"

        def rot(dst, src, n, psum_src=True, mult=1.0):
            eng = pick(n, psum_src, mult)
            if eng == "v":
                nc.vector.tensor_copy(dst, src)
            else:
                nc.scalar.copy(dst, src)

        with (
            tc.tile_pool(name="const", bufs=1) as constp,
            tc.tile_pool(name="wqkv", bufs=1) as wp,
            tc.tile_pool(name="acts", bufs=1) as actp,
            tc.tile_pool(name="mfp", bufs=2) as mfp,
            tc.tile_pool(name="sup", bufs=1) as sup,
            tc.tile_pool(name="vhrp", bufs=4) as vhrp,
            tc.tile_pool(name="obp", bufs=3) as obp,
            tc.tile_pool(name="sqp", bufs=3) as sqp,
            tc.tile_pool(name="recp", bufs=6) as recp,
            tc.tile_pool(name="ktpp", bufs=1, space="PSUM") as ktpp,
            tc.tile_pool(name="dlp", bufs=1, space="PSUM") as dlp,
            tc.tile_pool(name="pp", bufs=5, space="PSUM") as pp,
        ):
            consts = constp.tile([128, 644], BF)
            ident = consts[:, 0:128]
            maskf = consts[:, 128:512]  # [tri 128 | ones 128 | tri 128]
            onesrow64 = consts[64:65, 256:384]  # ones row [1,128] at partition 64
            hindt = consts[:, 512:514]  # per-head indicator [128,2]
            hindtT = consts[0:2, 514:642]  # transposed indicator [2,128]

            # warm up Act function tables off the critical path
            warm = actp.tile([1, 1], F32, tag="warm")
            nc.gpsimd.memset(warm[:], 1.0)
            warm2 = actp.tile([1, 1], F32, tag="warm2")
            nc.scalar.activation(warm2[:], warm[:], AF.Sqrt)

            # persistent activations
            qh2 = actp.tile([128, N], BF, tag="qh2")
            qh2b = actp.tile([64, N], BF, tag="qh2b")  # head-1 q at partitions 0:63
            kh2 = actp.tile([128, N], BF, tag="kh2")
            vcol = actp.tile([128, N], BF, tag="vcol")
            vht = actp.tile([128, N], BF, tag="vht")
            krows = actp.tile([128, NCH * RST], BF, tag="krows")
            vrows = actp.tile([128, NCH * RST], BF, tag="vrows")
            nrmbuf = actp.tile([2, 2 * NT], F32, tag="nrmbuf")
            scv128 = actp.tile([128, 1], F32, tag="scv128")
            sus = {}
            mfs = {}

            # transposes run in pairs through a 4-slot bf16 PSUM tile
            ktp = ktpp.tile([128, 768], BF, tag="tp", name="tp")
            tpctr = {"i": 0}

            # weights and X tiles, issued in first-use order; block-0 operands
            # ship in small pieces so the PE starts as early as possible
            wqA = wp.tile([128, 128], BF, tag="wqA")
            nc.sync.dma_start(wqA[:], wq_d[:, 0:128])
            xt = {}
            x0sl = [(0, 1), (1, 2), (2, 4), (4, 6), (6, 8)]
            for i, (ka, kb) in enumerate(x0sl):
                t = actp.tile([128, (kb - ka) * 512], BF, tag=f"x0_{i}", name=f"x0_{i}")
                xt[(0, i)] = t
                nc.sync.dma_start(t[:], xp_d[:, ka * 512 : kb * 512])
                if i == 0:
                    wqB = wp.tile([128, 896], BF, tag="wqB")
                    nc.sync.dma_start(wqB[:], wq_d[:, 128:1024])
                if i == 2:
                    wk_sb = wp.tile([128, D_MODEL], BF, tag="wk")
                    nc.sync.dma_start(wk_sb[:], wk_d[:])
                if i == 3:
                    nc.sync.dma_start(consts[:], consts_d[:])
            for n0 in range(1, NT):
                for hf in range(2):
                    t = actp.tile([128, 2048], BF, tag=f"x{n0}_{hf}", name=f"x{n0}_{hf}")
                    xt[(n0, hf)] = t
                    nc.sync.dma_start(
                        t[:], xp_d[:, n0 * 4096 + hf * 2048 : n0 * 4096 + (hf + 1) * 2048]
                    )
                if n0 == 1:
                    wv_sb = wp.tile([128, D_MODEL], BF, tag="wv")
                    nc.sync.dma_start(wv_sb[:], wv_d[:])
                if n0 == 2:
                    wo_sb = wp.tile([128, D_MODEL], BF, tag="wo")
                    nc.sync.dma_start(wo_sb[:], wo_d[:])

            def wq_ap(k):
                if k < 1:
                    return wqA[:]
                return wqB[:, (k - 1) * 128 : k * 128]

            def xap(k, n0):
                if n0 == 0:
                    for i, (ka, kb) in enumerate(x0sl):
                        if ka <= k < kb:
                            return xt[(0, i)][:, (k - ka) * 512 : (k - ka + 1) * 512]
                t = xt[(n0, k // 4)]
                return t[:, (k % 4) * 512 : (k % 4 + 1) * 512]

            # ones columns (64 and 129 of each chunk block) via full-tile fill
            nc.gpsimd.memset(krows[:], 1.0)
            nc.gpsimd.memset(vrows[:], 1.0)

            chains = {}

            def proj(n0, wap, dst, nm):
                p = pp.tile([128, 512], F32, tag="p", name=nm)
                for k in range(KT):
                    nc.tensor.matmul(
                        p[:], wap(k), xap(k, n0), start=(k == 0), stop=(k == KT - 1)
                    )
                rot(dst[:, n0 * 512 : (n0 + 1) * 512], p[:], 512)

            def sq_part(src, n0):
                cs = slice(n0 * 512, (n0 + 1) * 512)
                sq = sqp.tile([128, 512], BF, tag="sq", name="sq")
                nc.vector.tensor_mul(sq[:], src[:, cs], src[:, cs])
                load["v"] += _cost("v", 512, False, 0.5)
                return sq

            def nrm_part(j, n0, sq):
                nrmt = pp.tile([128, 512], F32, tag="p", name=f"nrm{j}")
                nc.tensor.matmul(nrmt[0:2, :], hindt, sq[:], start=True, stop=True)
                nc.vector.tensor_reduce(
                    nrmbuf[:, j * NT + n0 : j * NT + n0 + 1],
                    nrmt[0:2, :],
                    AX.X,
                    AluOpType.max,
                )
                load["v"] += _cost("v", 512)

            def sq_nrm(j, n0, src):
                # per-token norm^2 from the bf16 activations (cheap DVE square)
                nrm_part(j, n0, sq_part(src, n0))

            def trans_pair(src, rows, c0):
                # transpose chunks c0, c0+1 into an adjacent slot pair, then
                # evacuate both with ONE strided copy
                g = tpctr["i"] % 3
                tpctr["i"] += 1
                for u in range(2):
                    nc.tensor.transpose(
                        ktp[:, (2 * g + u) * 128 : (2 * g + u + 1) * 128],
                        src[:, (c0 + u) * CH : (c0 + u + 1) * CH],
                        ident,
                    )
                for u in range(2):
                    rdst = bass.AP(
                        rows[:].tensor,
                        rows[:].offset + (c0 + u) * RST,
                        [[NCH * RST, 128], [65, 2], [1, 64]],
                    )
                    rsrc = bass.AP(
                        ktp[:].tensor,
                        ktp[:].offset + (2 * g + u) * 128,
                        [[768, 128], [64, 2], [1, 64]],
                    )
                    rot(rdst, rsrc, 128, mult=0.5)

            def chain_su(sp):
                # prefix-state chain: one [65,65] PSUM tile per head
                ca, cb = 2 * sp, 2 * sp + 1
                sus[sp] = []
                for h in range(HPC):
                    if sp == 0:
                        chains[h] = dlp.tile(
                            [65, 65], F32, tag=f"chain{h}", name=f"chain{h}"
                        )
                    chn = chains[h]
                    for cc, st in ((ca, sp == 0), (cb, False)):
                        nc.tensor.matmul(
                            chn[:],
                            krows[:, cc * RST + h * 65 : cc * RST + h * 65 + 65],
                            vrows[:, cc * RST + h * 65 : cc * RST + h * 65 + 65],
                            start=st,
                            stop=(cc == cb),
                            skip_group_check=True,
                        )
                    su = sup.tile([65, 65], BF, tag=f"su{sp}_{h}", name=f"su{sp}_{h}")
                    rot(su[:], chn[:], 65)
                    sus[sp].append(su)

            def scores_mf(sp):
                mark(f"sp{sp}_scores")
                qs = slice(sp * SPAN, (sp + 1) * SPAN)
                cka = slice(sp * SPAN, sp * SPAN + CH)
                ckb = slice(sp * SPAN + CH, (sp + 1) * SPAN)
                for h in range(HPC):
                    hs = slice(h * 64, (h + 1) * 64)
                    ptj = pp.tile([128, 512], F32, tag="p", name="ptj")
                    nc.tensor.matmul(
                        ptj[:, 0:SPAN], kh2[hs, cka], qh2[hs, qs],
                        start=True, stop=True, tile_position=(h * 64, 0),
                    )
                    nc.tensor.matmul(
                        ptj[:, SPAN:384], kh2[hs, ckb], qh2[hs, ckb],
                        start=True, stop=True, tile_position=(h * 64, 0),
                    )
                    # mf = (ptj + a0) * [tri|ones|tri], one fused DVE op
                    mf = mfp.tile([128, 384], BF, tag=f"mf{h}", name=f"mf{h}")
                    nc.vector.scalar_tensor_tensor(
                        mf[:], ptj[:, 0:384], 1.0, maskf, AluOpType.add, AluOpType.mult
                    )
                    load["v"] += _cost("v", 384)
                    mfs[(sp, h)] = mf

            def attn_span(sp):
                # both heads share one [128,130] PSUM tile per chunk
                mark(f"sp{sp}_attn")
                ca, cb = 2 * sp, 2 * sp + 1
                vhr = vhrp.tile([128, 2 * CH], BF, tag="vhr", name="vhr")
                for cidx in (ca, cb):
                    ck = slice(cidx * CH, (cidx + 1) * CH)
                    o512 = pp.tile([128, 512], F32, tag="p", name="o")
                    o = o512[:, 0:130]
                    for h in range(HPC):
                        vra = vrows[:, ca * RST + h * 65 : ca * RST + h * 65 + 65]
                        vrb = vrows[:, cb * RST + h * 65 : cb * RST + h * 65 + 65]
                        mf = mfs[(sp, h)]
                        oh = o[:, h * 65 : (h + 1) * 65]
                        if cidx == ca:
                            parts = ((mf[:, 0:CH], vra),)
                        else:
                            parts = ((mf[:, CH : 2 * CH], vra), (mf[:, 2 * CH :], vrb))
                        for mi, (mm, vv) in enumerate(parts):
                            nc.tensor.matmul(
                                oh, mm, vv,
                                start=(mi == 0),
                                stop=(sp == 0 and mi == len(parts) - 1),
                                skip_group_check=True,
                            )
                        if sp > 0:
                            qsrc = qh2[0:64, ck] if h == 0 else qh2b[0:64, ck]
                            nc.tensor.matmul(
                                oh,
                                qsrc,
                                sus[sp - 1][h][0:64, :],
                                start=False,
                                stop=False,
                                skip_group_check=True,
                            )
                            nc.tensor.matmul(
                                oh,
                                onesrow64,
                                sus[sp - 1][h][64:65, :],
                                start=False,
                                stop=True,
                                tile_position=(64, 0),
                                skip_group_check=True,
                            )
                    # divide: rec = 1/den, vhr = num * rec (both heads at once)
                    rec = recp.tile([128, 2], F32, tag="rec", name="rec")
                    osrc = bass.AP(o512[:].tensor, o512[:].offset + 64, [[512, 128], [65, 2]])
                    nc.vector.reciprocal(rec[:], osrc)
                    load["v"] += _cost("v", 2)
                    half = (cidx - ca) * CH
                    vdst = bass.AP(
                        vhr[:].tensor,
                        vhr[:].offset + half,
                        [[2 * CH, 128], [64, 2], [1, 64]],
                    )
                    vsrc = bass.AP(
                        o512[:].tensor, o512[:].offset, [[512, 128], [65, 2], [1, 64]]
                    )
                    vrec = bass.AP(
                        rec[:].tensor, rec[:].offset, [[2, 128], [1, 2], [0, 64]]
                    )
                    nc.vector.tensor_mul(vdst, vsrc, vrec)
                    load["v"] += _cost("v", 128)
                return vhr

            def vht_span(sp, vhr):
                # transpose both chunks into an adjacent slot pair, evacuate
                # with ONE [128,256] copy into vht
                mark(f"sp{sp}_vht")
                g = tpctr["i"] % 3
                tpctr["i"] += 1
                for u in range(2):
                    nc.tensor.transpose(
                        ktp[:, (2 * g + u) * 128 : (2 * g + u + 1) * 128],
                        vhr[:, u * CH : (u + 1) * CH],
                        ident,
                    )
                rot(
                    vht[:, 2 * sp * CH : (2 * sp + 2) * CH],
                    ktp[:, 2 * g * 128 : (2 * g + 2) * 128],
                    256,
                    mult=0.5,
                )

            def outproj_chunk(cidx, ob, half, tail=False):
                ck = slice(cidx * CH, (cidx + 1) * CH)
                for n2 in range(D_MODEL // 512):
                    ns = slice(n2 * 512, (n2 + 1) * 512)
                    opp = pp.tile([128, 512], F32, tag="p", name="opp")
                    nc.tensor.matmul(
                        opp[:], vht[:, ck], wo_sb[:, ns], start=True, stop=True
                    )
                    cl = half * D_MODEL + n2 * 512
                    rot(ob[:, cl : cl + 512], opp[:], 512)
                    if tail:
                        nc.sync.dma_start(out_d[ck, ns], ob[:, cl : cl + 512])
                if not tail:
                    nc.sync.dma_start(
                        out_d[ck, :], ob[:, half * D_MODEL : (half + 1) * D_MODEL]
                    )

            # ====== stage A: q/k projections, norms, k transposes ======
            for n0 in range(NT):
                mark(f"blk{n0}_qk")
                proj(n0, wq_ap, qh2, "pq")
                if n0 > 0:
                    sq_nrm(1, n0 - 1, kh2)
                proj(n0, lambda k: wk_sb[:, k * 128 : (k + 1) * 128], kh2, "pk")
                sq_nrm(0, n0, qh2)
                if n0 > 0:
                    mark(f"blk{n0 - 1}_ktrans")
                    trans_pair(kh2, krows, 4 * (n0 - 1))
                    trans_pair(kh2, krows, 4 * (n0 - 1) + 2)
            mark("blk3_ktrans")
            trans_pair(kh2, krows, 4 * (NT - 1))
            trans_pair(kh2, krows, 4 * (NT - 1) + 2)
            sq_nrm(1, NT - 1, kh2)

            # norm finalize on DVE/Act while the PE streams v projections
            mq = actp.tile([2, 1], F32, tag="mq")
            mk = actp.tile([2, 1], F32, tag="mk")
            nc.vector.tensor_reduce(mq[:], nrmbuf[:, 0:NT], AX.X, AluOpType.max)
            nc.vector.tensor_reduce(mk[:], nrmbuf[:, NT : 2 * NT], AX.X, AluOpType.max)
            prod = actp.tile([2, 1], F32, tag="prod")
            nc.vector.tensor_mul(prod[:], mq[:], mk[:])
            rt = actp.tile([2, 1], F32, tag="rt")
            nc.scalar.activation(rt[:], prod[:], AF.Sqrt)
            rs = actp.tile([2, 1], F32, tag="rs")
            nc.vector.reciprocal(rs[:], rt[:])
            rsb = actp.tile([2, 1], BF, tag="rsb")
            nc.vector.tensor_copy(rsb[:], rs[:])

            # ====== stage A2: v projections + s broadcast + q scale ======
            mark("blk0_v")
            proj(0, lambda k: wv_sb[:, k * 128 : (k + 1) * 128], vcol, "pv")
            scvt = pp.tile([128, 512], F32, tag="p", name="scvt")
            nc.tensor.matmul(scvt[:, 0:1], hindtT, rsb[:], start=True, stop=True)
            nc.vector.tensor_copy(scv128[:], scvt[:, 0:1])
            nc.vector.tensor_scalar_mul(qh2[:], qh2[:], scv128[:])
            nc.vector.tensor_copy(qh2b[:], qh2[64:128, :])
            load["v"] += _cost("v", 2048, False, 0.25)
            for n0 in range(1, NT):
                mark(f"blk{n0}_v")
                proj(n0, lambda k: wv_sb[:, k * 128 : (k + 1) * 128], vcol, "pv")
                if n0 == 2:
                    scores_mf(0)
                if n0 >= 2:
                    mark(f"c{2 * (n0 - 2)}_vtrans0")
                    trans_pair(vcol, vrows, 2 * (n0 - 2))
            chain_su(0)
            if DBG == "qh2":
                nc.sync.dma_start(dbg_d[:, 0:N], qh2[:])
            if DBG == "kh2":
                nc.sync.dma_start(dbg_d[:, 0:N], kh2[:])

            # ====== stage B: v-trans / chain / scores / attn / outproj ======
            for sp in range(NSPAN):
                if sp == 0:
                    mark("blk0_vtrans")
                    trans_pair(vcol, vrows, 4)
                vhr = attn_span(sp)
                if sp < NSPAN - 1:
                    scores_mf(sp + 1)
                if sp > 0:
                    mark(f"sp{sp - 1}_outproj")
                    ob = obp.tile([128, 2 * D_MODEL], BF, tag="ob", name="ob")
                    outproj_chunk(2 * (sp - 1), ob, 0)
                if 0 < sp < NSPAN - 1:
                    chain_su(sp)
                if sp > 0:
                    outproj_chunk(2 * (sp - 1) + 1, ob, 1)
                vht_span(sp, vhr)
                if 2 * sp + 6 < NCH:
                    mark(f"c{2 * sp + 6}_vtrans")
                    trans_pair(vcol, vrows, 2 * sp + 6)
            mark(f"sp{NSPAN - 1}_outproj")
            obt = obp.tile([128, 2 * D_MODEL], BF, tag="ob", name="ob")
            outproj_chunk(2 * (NSPAN - 1), obt, 0, tail=True)
            outproj_chunk(2 * (NSPAN - 1) + 1, obt, 1, tail=True)
            if DBG == "vht":
                nc.sync.dma_start(dbg_d[:, 0:N], vht[:])
            if DBG == "vrows":
                nc.sync.dma_start(dbg_d[:, 0 : NCH * RST], vrows[:])
            if DBG == "krows":
                nc.sync.dma_start(dbg_d[:, 0 : NCH * RST], krows[:])
            if DBG == "su":
                for spd in range(NSPAN - 1):
                    nc.sync.dma_start(
                        dbg_d[0:65, spd * 65 : (spd + 1) * 65], sus[spd][0][:]
                    )
                    nc.sync.dma_start(
                        dbg_d[0:65, 2000 + spd * 65 : 2000 + (spd + 1) * 65],
                        sus[spd][1][:],
                    )

    nc.compile()
    return nc


def _consts():
    import ml_dtypes

    bf = ml_dtypes.bfloat16
    consts = np.zeros((128, 644), dtype=np.float32)
    consts[:, 0:128] = np.eye(128)
    j = np.arange(128)[:, None]
    i = np.arange(128)[None, :]
    tri = (j <= i).astype(np.float32)
    consts[:, 128:256] = tri
    consts[:, 256:384] = 1.0
    consts[:, 384:512] = tri
    for h in range(HPC):
        consts[h * 64 : (h + 1) * 64, 512 + h] = 1.0
        consts[h, 514 + h * 64 : 514 + (h + 1) * 64] = 1.0
    return consts.astype(bf)


def _in_maps(inputs):
    import ml_dtypes

    bf = ml_dtypes.bfloat16
    X = np.ascontiguousarray(np.asarray(inputs["X"], dtype=np.float32))
    xt = np.ascontiguousarray(X[0].T).astype(bf)  # [D_MODEL, N]
    # pack: [128, NT*KT*512], col n0*4096 + k*512 + c = X^T[k*128+p, n0*512+c]
    xp = np.ascontiguousarray(
        xt.reshape(KT, 128, NT, 512).transpose(1, 2, 0, 3).reshape(128, NT * KT * 512)
    )
    wqt = np.ascontiguousarray(np.asarray(inputs["Wq"], np.float32).T).astype(bf)
    wkt = np.ascontiguousarray(np.asarray(inputs["Wk"], np.float32).T).astype(bf)
    wvt = np.ascontiguousarray(np.asarray(inputs["Wv"], np.float32).T).astype(bf)
    wot = np.ascontiguousarray(np.asarray(inputs["Wo"], np.float32).T).astype(bf)
    consts = _consts()

    def sb_layout(w):  # [1024, 128] -> [128, 8*128] (dm-chunk on partitions)
        return np.ascontiguousarray(
            w.reshape(KT, 128, DPC).transpose(1, 0, 2).reshape(128, KT * DPC)
        )

    in_maps = []
    for c in range(NCORES):
        cs = slice(c * DPC, (c + 1) * DPC)
        in_maps.append(
            {
                "xp": xp,
                "wq": sb_layout(wqt[:, cs]),
                "wk": sb_layout(wkt[:, cs]),
                "wv": sb_layout(wvt[:, cs]),
                "wo": np.ascontiguousarray(wot[cs, :]),
                "consts": consts,
            }
        )
    return in_maps


def _run(inputs, trace=False):
    from concourse.bass_utils import run_bass_kernel_spmd

    if "nc" not in _CACHE:
        _CACHE["nc"] = _build()
    nc = _CACHE["nc"]
    in_maps = _in_maps(inputs)
    res = run_bass_kernel_spmd(nc, in_maps, core_ids=list(range(NCORES)), trace=trace)
    bo = np.asarray(inputs["bo"], dtype=np.float32)
    acc = np.zeros((N, D_MODEL), dtype=np.float32)
    for c in range(NCORES):
        acc += res.results[c]["out"].astype(np.float32)
    acc += bo[None, :]
    return acc.reshape(B, N, D_MODEL), res.exec_time_ns


def kernel(**inputs) -> np.ndarray:
    out, _ = _run(inputs, trace=False)
    return out


# revision 8
# speedup vs baseline: 1.0161x; 1.0161x over previous
"""Fastmax (p=1 causal linear attention) Trainium2 kernel, 8-core SPMD, v5.

Sharding: data-parallel over heads (16 heads -> 2 per core). Each core
computes q/k/v projections for its 2 heads, chunked causal linear attention
(augmented [65,65] prefix state per head), and a partial output projection;
the host sums the 8 partial outputs and adds the bias.

v5 schedule/copy-count rewrite:
  - One shared [128,512]-f32 PSUM ring (4 bufs) serves projections, norm
    rows, scores, attention o tiles and out-proj tiles; transposes run in a
    single [128,512]-bf16 4-slot tile in chunk PAIRS whose results leave in
    ONE strided copy per pair (half the transpose-evacuation ops).
  - Stage A: q/k projections + per-token norms + k transposes (PE-bound);
    the norm finalize runs on DVE/Act under the v projections; q is scaled
    once by s; scores then need no further scaling and a0 is folded into
    mf = (ptj + 1) * [tri|ones|tri] (one fused DVE op per head per span).
  - Stage B interleaves v transposes (two chunks ahead), the prefix-state
    chain, scores, attention, and the one-span-delayed output projection.
  - State snapshots are ONE [65,65] copy per head (base partition 0); the
    head-1 q rows are replicated to partitions 0:63 once (qh2b) so the
    state matmuls' operands share base partitions; the prefix ones-row term
    uses a ones lhsT at partition 64 matching the snapshot row.
"""

import sys

sys.path.insert(0, "/opt/trn_rl_repo")

import numpy as np

B, N, D_MODEL, H, D_HEAD = 1, 2048, 1024, 16, 64
NCORES = 8
HPC = H // NCORES  # heads per core
DPC = HPC * D_HEAD  # out dims per core (128)
CH = 128  # chunk (tokens)
SPAN = 256  # query span (2 chunks)
NSPAN = N // SPAN
NCH = N // CH
KT = D_MODEL // 128  # contraction tiles for projections
NT = N // 512  # 512-wide column chunks of the sequence
RST = 136  # row-buffer stride per chunk (2*(64 data + ones col) + pad)

_CACHE = {}
DBG = None
MARKS = []


def _build():
    import concourse.bass as bass
    import concourse.tile as tile
    import concourse.mybir as mybir
    from concourse import bacc
    from concourse.alu_op_type import AluOpType

    BF = mybir.dt.bfloat16
    F32 = mybir.dt.float32
    AF = mybir.ActivationFunctionType
    AX = mybir.AxisListType

    nc = bacc.Bacc("TRN2", target_bir_lowering=False, debug=False, num_devices=NCORES)

    # xp: host-packed X, col n0*4096 + k*512 + c <-> X[tok n0*512+c, dm k*128+p]
    xp_d = nc.declare_dram_parameter("xp", [128, NT * KT * 512], BF, isOutput=False)
    wq_d = nc.declare_dram_parameter("wq", [128, D_MODEL], BF, isOutput=False)
    wk_d = nc.declare_dram_parameter("wk", [128, D_MODEL], BF, isOutput=False)
    wv_d = nc.declare_dram_parameter("wv", [128, D_MODEL], BF, isOutput=False)
    wo_d = nc.declare_dram_parameter("wo", [DPC, D_MODEL], BF, isOutput=False)
    consts_d = nc.declare_dram_parameter("consts", [128, 644], BF, isOutput=False)
    out_d = nc.declare_dram_parameter("out", [N, D_MODEL], BF, isOutput=True)
    dbg_d = nc.declare_dram_parameter("dbg", [128, 4352], BF, isOutput=True) if DBG else None

    # engine load balancer for PSUM->SBUF copies & small ops (cost-model based)
    load = {"v": 0.0, "s": 0.0}

    def _cost(eng, n, psum_src=True, mult=1.0):
        if eng == "v":
            return n * 1.04 * mult + (125.0 if psum_src else 60.0)
        return n * 0.92 + (160.0 if psum_src else 200.0)

    def mark(label):
        MARKS.append((label, int(nc.get_next_instruction_name()[2:])))

    with tile.TileContext(nc) as tc:

        def pick(n, psum_src=True, mult=1.0):
            cv = load["v"] + _cost("v", n, psum_src, mult)
            cs = load["s"] + _cost("s", n, psum_src)
            if cv <= cs:
                load["v"] = cv
                return "v"
            load["s"] = cs
            return "s"

        def rot(dst, src, n, psum_src=True, mult=1.0):
            eng = pick(n, psum_src, mult)
            if eng == "v":
                nc.vector.tensor_copy(dst, src)
            else:
                nc.scalar.copy(dst, src)

        with (
            tc.tile_pool(name="const", bufs=1) as constp,
            tc.tile_pool(name="wqkv", bufs=1) as wp,
            tc.tile_pool(name="acts", bufs=1) as actp,
            tc.tile_pool(name="mfp", bufs=2) as mfp,
            tc.tile_pool(name="sup", bufs=1) as sup,
            tc.tile_pool(name="vhrp", bufs=4) as vhrp,
            tc.tile_pool(name="obp", bufs=3) as obp,
            tc.tile_pool(name="sqp", bufs=3) as sqp,
            tc.tile_pool(name="recp", bufs=6) as recp,
            tc.tile_pool(name="ktpp", bufs=1, space="PSUM") as ktpp,
            tc.tile_pool(name="dlp", bufs=1, space="PSUM") as dlp,
            tc.tile_pool(name="pp", bufs=5, space="PSUM") as pp,
        ):
            consts = constp.tile([128, 644], BF)
            ident = consts[:, 0:128]
            maskf = consts[:, 128:512]  # [tri 128 | ones 128 | tri 128]
            onesrow64 = consts[64:65, 256:384]  # ones row [1,128] at partition 64
            hindt = consts[:, 512:514]  # per-head indicator [128,2]
            hindtT = consts[0:2, 514:642]  # transposed indicator [2,128]

            # warm up Act function tables off the critical path
            warm = actp.tile([1, 1], F32, tag="warm")
            nc.gpsimd.memset(warm[:], 1.0)
            warm2 = actp.tile([1, 1], F32, tag="warm2")
            nc.scalar.activation(warm2[:], warm[:], AF.Sqrt)

            # persistent activations
            qh2 = actp.tile([128, N], BF, tag="qh2")
            qh2b = actp.tile([64, N], BF, tag="qh2b")  # head-1 q at partitions 0:63
            kh2 = actp.tile([128, N], BF, tag="kh2")
            vcol = actp.tile([128, N], BF, tag="vcol")
            vht = actp.tile([128, N], BF, tag="vht")
            krows = actp.tile([128, NCH * RST], BF, tag="krows")
            vrows = actp.tile([128, NCH * RST], BF, tag="vrows")
            nrmbuf = actp.tile([2, 2 * NT], F32, tag="nrmbuf")
            scv128 = actp.tile([128, 1], F32, tag="scv128")
            sus = {}
            mfs = {}

            # transposes run in pairs through a 4-slot bf16 PSUM tile
            ktp = ktpp.tile([128, 1024], BF, tag="tp", name="tp")
            tpctr = {"i": 0}

            # weights and X tiles, issued in first-use order; block-0 operands
            # ship in small pieces so the PE starts as early as possible
            wqA = wp.tile([128, 128], BF, tag="wqA")
            nc.sync.dma_start(wqA[:], wq_d[:, 0:128])
            xt = {}
            x0sl = [(0, 1), (1, 2), (2, 4), (4, 6), (6, 8)]
            for i, (ka, kb) in enumerate(x0sl):
                t = actp.tile([128, (kb - ka) * 512], BF, tag=f"x0_{i}", name=f"x0_{i}")
                xt[(0, i)] = t
                nc.sync.dma_start(t[:], xp_d[:, ka * 512 : kb * 512])
                if i == 0:
                    wqB = wp.tile([128, 896], BF, tag="wqB")
                    nc.sync.dma_start(wqB[:], wq_d[:, 128:1024])
                if i == 2:
                    wk_sb = wp.tile([128, D_MODEL], BF, tag="wk")
                    nc.sync.dma_start(wk_sb[:], wk_d[:])
                if i == 3:
                    nc.sync.dma_start(consts[:], consts_d[:])
            for n0 in range(1, NT):
                for hf in range(2):
                    t = actp.tile([128, 2048], BF, tag=f"x{n0}_{hf}", name=f"x{n0}_{hf}")
                    xt[(n0, hf)] = t
                    nc.sync.dma_start(
                        t[:], xp_d[:, n0 * 4096 + hf * 2048 : n0 * 4096 + (hf + 1) * 2048]
                    )
                if n0 == 1:
                    wv_sb = wp.tile([128, D_MODEL], BF, tag="wv")
                    nc.sync.dma_start(wv_sb[:], wv_d[:])
                if n0 == 2:
                    wo_sb = wp.tile([128, D_MODEL], BF, tag="wo")
                    nc.sync.dma_start(wo_sb[:], wo_d[:])

            def wq_ap(k):
                if k < 1:
                    return wqA[:]
                return wqB[:, (k - 1) * 128 : k * 128]

            def xap(k, n0):
                if n0 == 0:
                    for i, (ka, kb) in enumerate(x0sl):
                        if ka <= k < kb:
                            return xt[(0, i)][:, (k - ka) * 512 : (k - ka + 1) * 512]
                t = xt[(n0, k // 4)]
                return t[:, (k % 4) * 512 : (k % 4 + 1) * 512]

            # ones columns (64 and 129 of each chunk block) via full-tile fill
            nc.gpsimd.memset(krows[:], 1.0)
            nc.gpsimd.memset(vrows[:], 1.0)

            chains = {}

            def proj(n0, wap, dst, nm):
                p = pp.tile([128, 512], F32, tag="p", name=nm)
                for k in range(KT):
                    nc.tensor.matmul(
                        p[:], wap(k), xap(k, n0), start=(k == 0), stop=(k == KT - 1)
                    )
                rot(dst[:, n0 * 512 : (n0 + 1) * 512], p[:], 512)

            def sq_part(src, n0):
                cs = slice(n0 * 512, (n0 + 1) * 512)
                sq = sqp.tile([128, 512], BF, tag="sq", name="sq")
                nc.vector.tensor_mul(sq[:], src[:, cs], src[:, cs])
                load["v"] += _cost("v", 512, False, 0.5)
                return sq

            def nrm_part(j, n0, sq):
                nrmt = pp.tile([128, 512], F32, tag="p", name=f"nrm{j}")
                nc.tensor.matmul(nrmt[0:2, :], hindt, sq[:], start=True, stop=True)
                nc.vector.tensor_reduce(
                    nrmbuf[:, j * NT + n0 : j * NT + n0 + 1],
                    nrmt[0:2, :],
                    AX.X,
                    AluOpType.max,
                )
                load["v"] += _cost("v", 512)

            def sq_nrm(j, n0, src):
                # per-token norm^2 from the bf16 activations (cheap DVE square)
                nrm_part(j, n0, sq_part(src, n0))

            def trans_pair(src, rows, c0):
                # transpose chunks c0, c0+1 into an adjacent slot pair, then
                # evacuate both with ONE strided copy
                g = tpctr["i"] % 4
                tpctr["i"] += 1
                for u in range(2):
                    nc.tensor.transpose(
                        ktp[:, (2 * g + u) * 128 : (2 * g + u + 1) * 128],
                        src[:, (c0 + u) * CH : (c0 + u + 1) * CH],
                        ident,
                    )
                for u in range(2):
                    rdst = bass.AP(
                        rows[:].tensor,
                        rows[:].offset + (c0 + u) * RST,
                        [[NCH * RST, 128], [65, 2], [1, 64]],
                    )
                    rsrc = bass.AP(
                        ktp[:].tensor,
                        ktp[:].offset + (2 * g + u) * 128,
                        [[1024, 128], [64, 2], [1, 64]],
                    )
                    rot(rdst, rsrc, 128, mult=0.5)

            def chain_su(sp):
                # prefix-state chain: one [65,65] PSUM tile per head
                ca, cb = 2 * sp, 2 * sp + 1
                sus[sp] = []
                for h in range(HPC):
                    if sp == 0:
                        chains[h] = dlp.tile(
                            [65, 65], F32, tag=f"chain{h}", name=f"chain{h}"
                        )
                    chn = chains[h]
                    for cc, st in ((ca, sp == 0), (cb, False)):
                        nc.tensor.matmul(
                            chn[:],
                            krows[:, cc * RST + h * 65 : cc * RST + h * 65 + 65],
                            vrows[:, cc * RST + h * 65 : cc * RST + h * 65 + 65],
                            start=st,
                            stop=(cc == cb),
                            skip_group_check=True,
                        )
                    su = sup.tile([65, 65], BF, tag=f"su{sp}_{h}", name=f"su{sp}_{h}")
                    rot(su[:], chn[:], 65)
                    sus[sp].append(su)

            def scores_mf(sp):
                mark(f"sp{sp}_scores")
                qs = slice(sp * SPAN, (sp + 1) * SPAN)
                cka = slice(sp * SPAN, sp * SPAN + CH)
                ckb = slice(sp * SPAN + CH, (sp + 1) * SPAN)
                for h in range(HPC):
                    hs = slice(h * 64, (h + 1) * 64)
                    ptj = pp.tile([128, 512], F32, tag="p", name="ptj")
                    nc.tensor.matmul(
                        ptj[:, 0:SPAN], kh2[hs, cka], qh2[hs, qs],
                        start=True, stop=True, tile_position=(h * 64, 0),
                    )
                    nc.tensor.matmul(
                        ptj[:, SPAN:384], kh2[hs, ckb], qh2[hs, ckb],
                        start=True, stop=True, tile_position=(h * 64, 0),
                    )
                    # mf = (ptj + a0) * [tri|ones|tri], one fused DVE op
                    mf = mfp.tile([128, 384], BF, tag=f"mf{h}", name=f"mf{h}")
                    nc.vector.scalar_tensor_tensor(
                        mf[:], ptj[:, 0:384], 1.0, maskf, AluOpType.add, AluOpType.mult
                    )
                    load["v"] += _cost("v", 384)
                    mfs[(sp, h)] = mf

            def attn_chunk(sp, cidx, vhr):
                ca, cb = 2 * sp, 2 * sp + 1
                if True:
                    ck = slice(cidx * CH, (cidx + 1) * CH)
                    o512 = pp.tile([128, 512], F32, tag="p", name="o")
                    o = o512[:, 0:130]
                    for h in range(HPC):
                        vra = vrows[:, ca * RST + h * 65 : ca * RST + h * 65 + 65]
                        vrb = vrows[:, cb * RST + h * 65 : cb * RST + h * 65 + 65]
                        mf = mfs[(sp, h)]
                        oh = o[:, h * 65 : (h + 1) * 65]
                        if cidx == ca:
                            parts = ((mf[:, 0:CH], vra),)
                        else:
                            parts = ((mf[:, CH : 2 * CH], vra), (mf[:, 2 * CH :], vrb))
                        for mi, (mm, vv) in enumerate(parts):
                            nc.tensor.matmul(
                                oh, mm, vv,
                                start=(mi == 0),
                                stop=(sp == 0 and mi == len(parts) - 1),
                                skip_group_check=True,
                            )
                        if sp > 0:
                            qsrc = qh2[0:64, ck] if h == 0 else qh2b[0:64, ck]
                            nc.tensor.matmul(
                                oh,
                                qsrc,
                                sus[sp - 1][h][0:64, :],
                                start=False,
                                stop=False,
                                skip_group_check=True,
                            )
                            nc.tensor.matmul(
                                oh,
                                onesrow64,
                                sus[sp - 1][h][64:65, :],
                                start=False,
                                stop=True,
                                tile_position=(64, 0),
                                skip_group_check=True,
                            )
                    # divide: rec = 1/den, vhr = num * rec (both heads at once)
                    rec = recp.tile([128, 2], F32, tag="rec", name="rec")
                    osrc = bass.AP(o512[:].tensor, o512[:].offset + 64, [[512, 128], [65, 2]])
                    nc.vector.reciprocal(rec[:], osrc)
                    load["v"] += _cost("v", 2)
                    half = (cidx - ca) * CH
                    vdst = bass.AP(
                        vhr[:].tensor,
                        vhr[:].offset + half,
                        [[2 * CH, 128], [64, 2], [1, 64]],
                    )
                    vsrc = bass.AP(
                        o512[:].tensor, o512[:].offset, [[512, 128], [65, 2], [1, 64]]
                    )
                    vrec = bass.AP(
                        rec[:].tensor, rec[:].offset, [[2, 128], [1, 2], [0, 64]]
                    )
                    nc.vector.tensor_mul(vdst, vsrc, vrec)
                    load["v"] += _cost("v", 128)

            def attn_span(sp):
                # both heads share one [128,130] PSUM tile per chunk
                mark(f"sp{sp}_attn")
                vhr = vhrp.tile([128, 2 * CH], BF, tag="vhr", name="vhr")
                attn_chunk(sp, 2 * sp, vhr)
                attn_chunk(sp, 2 * sp + 1, vhr)
                return vhr

            def vht_half(sp, vhr, half):
                g = tpctr["i"] % 4
                tpctr["i"] += 1
                sl = 2 * g * 128
                nc.tensor.transpose(
                    ktp[:, sl : sl + 128], vhr[:, half * CH : (half + 1) * CH], ident
                )
                rot(
                    vht[:, (2 * sp + half) * CH : (2 * sp + half + 1) * CH],
                    ktp[:, sl : sl + 128],
                    128,
                    mult=0.5,
                )

            def vht_span(sp, vhr):
                # transpose both chunks into an adjacent slot pair, evacuate
                # with ONE [128,256] copy into vht
                mark(f"sp{sp}_vht")
                g = tpctr["i"] % 4
                tpctr["i"] += 1
                for u in range(2):
                    nc.tensor.transpose(
                        ktp[:, (2 * g + u) * 128 : (2 * g + u + 1) * 128],
                        vhr[:, u * CH : (u + 1) * CH],
                        ident,
                    )
                rot(
                    vht[:, 2 * sp * CH : (2 * sp + 2) * CH],
                    ktp[:, 2 * g * 128 : (2 * g + 2) * 128],
                    256,
                    mult=0.5,
                )

            def outproj_chunk(cidx, ob, half, tail=False):
                ck = slice(cidx * CH, (cidx + 1) * CH)
                for n2 in range(D_MODEL // 512):
                    ns = slice(n2 * 512, (n2 + 1) * 512)
                    opp = pp.tile([128, 512], F32, tag="p", name="opp")
                    nc.tensor.matmul(
                        opp[:], vht[:, ck], wo_sb[:, ns], start=True, stop=True
                    )
                    cl = half * D_MODEL + n2 * 512
                    rot(ob[:, cl : cl + 512], opp[:], 512)
                    if tail:
                        nc.sync.dma_start(out_d[ck, ns], ob[:, cl : cl + 512])
                if not tail:
                    nc.sync.dma_start(
                        out_d[ck, :], ob[:, half * D_MODEL : (half + 1) * D_MODEL]
                    )

            # ====== stage A: q/k projections, norms, k transposes ======
            for n0 in range(NT):
                mark(f"blk{n0}_qk")
                proj(n0, wq_ap, qh2, "pq")
                if n0 > 0:
                    sq_nrm(1, n0 - 1, kh2)
                proj(n0, lambda k: wk_sb[:, k * 128 : (k + 1) * 128], kh2, "pk")
                sq_nrm(0, n0, qh2)
                if n0 > 0:
                    mark(f"blk{n0 - 1}_ktrans")
                    trans_pair(kh2, krows, 4 * (n0 - 1))
                    trans_pair(kh2, krows, 4 * (n0 - 1) + 2)
            mark("blk3_ktrans")
            trans_pair(kh2, krows, 4 * (NT - 1))
            trans_pair(kh2, krows, 4 * (NT - 1) + 2)
            sq_nrm(1, NT - 1, kh2)

            # norm finalize on DVE/Act while the PE streams v projections
            mq = actp.tile([2, 1], F32, tag="mq")
            mk = actp.tile([2, 1], F32, tag="mk")
            nc.vector.tensor_reduce(mq[:], nrmbuf[:, 0:NT], AX.X, AluOpType.max)
            nc.vector.tensor_reduce(mk[:], nrmbuf[:, NT : 2 * NT], AX.X, AluOpType.max)
            prod = actp.tile([2, 1], F32, tag="prod")
            nc.vector.tensor_mul(prod[:], mq[:], mk[:])
            rt = actp.tile([2, 1], F32, tag="rt")
            nc.scalar.activation(rt[:], prod[:], AF.Sqrt)
            rs = actp.tile([2, 1], F32, tag="rs")
            nc.vector.reciprocal(rs[:], rt[:])
            rsb = actp.tile([2, 1], BF, tag="rsb")
            nc.vector.tensor_copy(rsb[:], rs[:])

            # ====== stage A2: v projections + s broadcast + q scale ======
            mark("blk0_v")
            proj(0, lambda k: wv_sb[:, k * 128 : (k + 1) * 128], vcol, "pv")
            scvt = pp.tile([128, 512], F32, tag="p", name="scvt")
            nc.tensor.matmul(scvt[:, 0:1], hindtT, rsb[:], start=True, stop=True)
            nc.vector.tensor_copy(scv128[:], scvt[:, 0:1])
            nc.vector.tensor_scalar_mul(qh2[:], qh2[:], scv128[:])
            nc.vector.tensor_copy(qh2b[:], qh2[64:128, :])
            load["v"] += _cost("v", 2048, False, 0.25)
            for n0 in range(1, NT):
                mark(f"blk{n0}_v")
                proj(n0, lambda k: wv_sb[:, k * 128 : (k + 1) * 128], vcol, "pv")
                if n0 == 2:
                    scores_mf(0)
                if n0 >= 2:
                    mark(f"c{2 * (n0 - 2)}_vtrans0")
                    trans_pair(vcol, vrows, 2 * (n0 - 2))
            chain_su(0)
            if DBG == "qh2":
                nc.sync.dma_start(dbg_d[:, 0:N], qh2[:])
            if DBG == "kh2":
                nc.sync.dma_start(dbg_d[:, 0:N], kh2[:])

            # ====== stage B: v-trans / chain / scores / attn / outproj ======
            for sp in range(NSPAN):
                if sp == 0:
                    mark("blk0_vtrans")
                    trans_pair(vcol, vrows, 4)
                vhr = attn_span(sp)
                if sp < NSPAN - 1:
                    scores_mf(sp + 1)
                if sp > 0:
                    mark(f"sp{sp - 1}_outproj")
                    ob = obp.tile([128, 2 * D_MODEL], BF, tag="ob", name="ob")
                    outproj_chunk(2 * (sp - 1), ob, 0)
                if 0 < sp < NSPAN - 1:
                    chain_su(sp)
                if sp > 0:
                    outproj_chunk(2 * (sp - 1) + 1, ob, 1)
                if sp < NSPAN - 1:
                    vht_span(sp, vhr)
                else:
                    vhr_last = vhr
                if 2 * sp + 6 < NCH:
                    mark(f"c{2 * sp + 6}_vtrans")
                    trans_pair(vcol, vrows, 2 * sp + 6)
            mark(f"sp{NSPAN - 1}_outproj")
            obt = obp.tile([128, 2 * D_MODEL], BF, tag="ob", name="ob")
            vht_half(NSPAN - 1, vhr_last, 0)
            outproj_chunk(2 * (NSPAN - 1), obt, 0, tail=True)
            vht_half(NSPAN - 1, vhr_last, 1)
            outproj_chunk(2 * (NSPAN - 1) + 1, obt, 1, tail=True)
            if DBG == "vht":
                nc.sync.dma_start(dbg_d[:, 0:N], vht[:])
            if DBG == "vrows":
                nc.sync.dma_start(dbg_d[:, 0 : NCH * RST], vrows[:])
            if DBG == "krows":
                nc.sync.dma_start(dbg_d[:, 0 : NCH * RST], krows[:])
            if DBG == "su":
                for spd in range(NSPAN - 1):
                    nc.sync.dma_start(
                        dbg_d[0:65, spd * 65 : (spd + 1) * 65], sus[spd][0][:]
                    )
                    nc.sync.dma_start(
                        dbg_d[0:65, 2000 + spd * 65 : 2000 + (spd + 1) * 65],
                        sus[spd][1][:],
                    )

    nc.compile()
    return nc


def _consts():
    import ml_dtypes

    bf = ml_dtypes.bfloat16
    consts = np.zeros((128, 644), dtype=np.float32)
    consts[:, 0:128] = np.eye(128)
    j = np.arange(128)[:, None]
    i = np.arange(128)[None, :]
    tri = (j <= i).astype(np.float32)
    consts[:, 128:256] = tri
    consts[:, 256:384] = 1.0
    consts[:, 384:512] = tri
    for h in range(HPC):
        consts[h * 64 : (h + 1) * 64, 512 + h] = 1.0
        consts[h, 514 + h * 64 : 514 + (h + 1) * 64] = 1.0
    return consts.astype(bf)


def _in_maps(inputs):
    import ml_dtypes

    bf = ml_dtypes.bfloat16
    X = np.ascontiguousarray(np.asarray(inputs["X"], dtype=np.float32))
    xt = np.ascontiguousarray(X[0].T).astype(bf)  # [D_MODEL, N]
    # pack: [128, NT*KT*512], col n0*4096 + k*512 + c = X^T[k*128+p, n0*512+c]
    xp = np.ascontiguousarray(
        xt.reshape(KT, 128, NT, 512).transpose(1, 2, 0, 3).reshape(128, NT * KT * 512)
    )
    wqt = np.ascontiguousarray(np.asarray(inputs["Wq"], np.float32).T).astype(bf)
    wkt = np.ascontiguousarray(np.asarray(inputs["Wk"], np.float32).T).astype(bf)
    wvt = np.ascontiguousarray(np.asarray(inputs["Wv"], np.float32).T).astype(bf)
    wot = np.ascontiguousarray(np.asarray(inputs["Wo"], np.float32).T).astype(bf)
    consts = _consts()

    def sb_layout(w):  # [1024, 128] -> [128, 8*128] (dm-chunk on partitions)
        return np.ascontiguousarray(
            w.reshape(KT, 128, DPC).transpose(1, 0, 2).reshape(128, KT * DPC)
        )

    in_maps = []
    for c in range(NCORES):
        cs = slice(c * DPC, (c + 1) * DPC)
        in_maps.append(
            {
                "xp": xp,
                "wq": sb_layout(wqt[:, cs]),
                "wk": sb_layout(wkt[:, cs]),
                "wv": sb_layout(wvt[:, cs]),
                "wo": np.ascontiguousarray(wot[cs, :]),
                "consts": consts,
            }
        )
    return in_maps


def _run(inputs, trace=False):
    from concourse.bass_utils import run_bass_kernel_spmd

    if "nc" not in _CACHE:
        _CACHE["nc"] = _build()
    nc = _CACHE["nc"]
    in_maps = _in_maps(inputs)
    res = run_bass_kernel_spmd(nc, in_maps, core_ids=list(range(NCORES)), trace=trace)
    bo = np.asarray(inputs["bo"], dtype=np.float32)
    acc = np.zeros((N, D_MODEL), dtype=np.float32)
    for c in range(NCORES):
        acc += res.results[c]["out"].astype(np.float32)
    acc += bo[None, :]
    return acc.reshape(B, N, D_MODEL), res.exec_time_ns


def kernel(**inputs) -> np.ndarray:
    out, _ = _run(inputs, trace=False)
    return out


# revision 9
# speedup vs baseline: 1.0263x; 1.0101x over previous
"""Fastmax (p=1 causal linear attention) Trainium2 kernel, 8-core SPMD, v5.

Sharding: data-parallel over heads (16 heads -> 2 per core). Each core
computes q/k/v projections for its 2 heads, chunked causal linear attention
(augmented [65,65] prefix state per head), and a partial output projection;
the host sums the 8 partial outputs and adds the bias.

v5 schedule/copy-count rewrite:
  - One shared [128,512]-f32 PSUM ring (4 bufs) serves projections, norm
    rows, scores, attention o tiles and out-proj tiles; transposes run in a
    single [128,512]-bf16 4-slot tile in chunk PAIRS whose results leave in
    ONE strided copy per pair (half the transpose-evacuation ops).
  - Stage A: q/k projections + per-token norms + k transposes (PE-bound);
    the norm finalize runs on DVE/Act under the v projections; q is scaled
    once by s; scores then need no further scaling and a0 is folded into
    mf = (ptj + 1) * [tri|ones|tri] (one fused DVE op per head per span).
  - Stage B interleaves v transposes (two chunks ahead), the prefix-state
    chain, scores, attention, and the one-span-delayed output projection.
  - State snapshots are ONE [65,65] copy per head (base partition 0); the
    head-1 q rows are replicated to partitions 0:63 once (qh2b) so the
    state matmuls' operands share base partitions; the prefix ones-row term
    uses a ones lhsT at partition 64 matching the snapshot row.
"""

import sys

sys.path.insert(0, "/opt/trn_rl_repo")

import numpy as np

B, N, D_MODEL, H, D_HEAD = 1, 2048, 1024, 16, 64
NCORES = 8
HPC = H // NCORES  # heads per core
DPC = HPC * D_HEAD  # out dims per core (128)
CH = 128  # chunk (tokens)
SPAN = 256  # query span (2 chunks)
NSPAN = N // SPAN
NCH = N // CH
KT = D_MODEL // 128  # contraction tiles for projections
NT = N // 512  # 512-wide column chunks of the sequence
RST = 136  # row-buffer stride per chunk (2*(64 data + ones col) + pad)

_CACHE = {}
DBG = None
MARKS = []


def _build():
    import concourse.bass as bass
    import concourse.tile as tile
    import concourse.mybir as mybir
    from concourse import bacc
    from concourse.alu_op_type import AluOpType

    BF = mybir.dt.bfloat16
    F32 = mybir.dt.float32
    AF = mybir.ActivationFunctionType
    AX = mybir.AxisListType

    nc = bacc.Bacc("TRN2", target_bir_lowering=False, debug=False, num_devices=NCORES)

    # xp: host-packed X, col n0*4096 + k*512 + c <-> X[tok n0*512+c, dm k*128+p]
    xp_d = nc.declare_dram_parameter("xp", [128, NT * KT * 512], BF, isOutput=False)
    wq_d = nc.declare_dram_parameter("wq", [128, D_MODEL], BF, isOutput=False)
    wk_d = nc.declare_dram_parameter("wk", [128, D_MODEL], BF, isOutput=False)
    wv_d = nc.declare_dram_parameter("wv", [128, D_MODEL], BF, isOutput=False)
    wo_d = nc.declare_dram_parameter("wo", [DPC, D_MODEL], BF, isOutput=False)
    consts_d = nc.declare_dram_parameter("consts", [128, 644], BF, isOutput=False)
    out_d = nc.declare_dram_parameter("out", [N, D_MODEL], BF, isOutput=True)
    dbg_d = nc.declare_dram_parameter("dbg", [128, 4352], BF, isOutput=True) if DBG else None

    # engine load balancer for PSUM->SBUF copies & small ops (cost-model based)
    load = {"v": 0.0, "s": 0.0}

    def _cost(eng, n, psum_src=True, mult=1.0):
        if eng == "v":
            return n * 1.04 * mult + (125.0 if psum_src else 60.0)
        return n * 0.92 + (160.0 if psum_src else 200.0)

    def mark(label):
        MARKS.append((label, int(nc.get_next_instruction_name()[2:])))

    with tile.TileContext(nc) as tc:

        def pick(n, psum_src=True, mult=1.0):
            cv = load["v"] + _cost("v", n, psum_src, mult)
            cs = load["s"] + _cost("s", n, psum_src)
            if cv <= cs:
                load["v"] = cv
                return "v"
            load["s"] = cs
            return "s"

        def rot(dst, src, n, psum_src=True, mult=1.0):
            eng = pick(n, psum_src, mult)
            if eng == "v":
                nc.vector.tensor_copy(dst, src)
            else:
                nc.scalar.copy(dst, src)

        with (
            tc.tile_pool(name="const", bufs=1) as constp,
            tc.tile_pool(name="wqkv", bufs=1) as wp,
            tc.tile_pool(name="acts", bufs=1) as actp,
            tc.tile_pool(name="mfp", bufs=2) as mfp,
            tc.tile_pool(name="sup", bufs=1) as sup,
            tc.tile_pool(name="vhrp", bufs=4) as vhrp,
            tc.tile_pool(name="obp", bufs=3) as obp,
            tc.tile_pool(name="sqp", bufs=3) as sqp,
            tc.tile_pool(name="recp", bufs=6) as recp,
            tc.tile_pool(name="ktpp", bufs=1, space="PSUM") as ktpp,
            tc.tile_pool(name="dlp", bufs=1, space="PSUM") as dlp,
            tc.tile_pool(name="pp", bufs=5, space="PSUM") as pp,
        ):
            consts = constp.tile([128, 644], BF)
            ident = consts[:, 0:128]
            maskf = consts[:, 128:512]  # [tri 128 | ones 128 | tri 128]
            onesrow64 = consts[64:65, 256:384]  # ones row [1,128] at partition 64
            hindt = consts[:, 512:514]  # per-head indicator [128,2]
            hindtT = consts[0:2, 514:642]  # transposed indicator [2,128]

            # warm up Act function tables off the critical path
            warm = actp.tile([1, 1], F32, tag="warm")
            nc.gpsimd.memset(warm[:], 1.0)
            warm2 = actp.tile([1, 1], F32, tag="warm2")
            nc.scalar.activation(warm2[:], warm[:], AF.Sqrt)

            # persistent activations
            qh2 = actp.tile([128, N], BF, tag="qh2")
            qh2b = actp.tile([64, N], BF, tag="qh2b")  # head-1 q at partitions 0:63
            kh2 = actp.tile([128, N], BF, tag="kh2")
            vcol = actp.tile([128, N], BF, tag="vcol")
            vht = actp.tile([128, N], BF, tag="vht")
            krows = actp.tile([128, NCH * RST], BF, tag="krows")
            vrows = actp.tile([128, NCH * RST], BF, tag="vrows")
            nrmbuf = actp.tile([2, 2 * NT], F32, tag="nrmbuf")
            scv128 = actp.tile([128, 1], F32, tag="scv128")
            sus = {}
            mfs = {}

            # transposes run in pairs through a 4-slot bf16 PSUM tile
            ktp = ktpp.tile([128, 1024], BF, tag="tp", name="tp")
            tpctr = {"i": 0}

            # weights and X tiles, issued in first-use order; block-0 operands
            # ship in small pieces so the PE starts as early as possible
            wqA = wp.tile([128, 128], BF, tag="wqA")
            nc.sync.dma_start(wqA[:], wq_d[:, 0:128])
            xt = {}
            x0sl = [(0, 4), (4, 8)]
            for i, (ka, kb) in enumerate(x0sl):
                t = actp.tile([128, (kb - ka) * 512], BF, tag=f"x0_{i}", name=f"x0_{i}")
                xt[(0, i)] = t
                nc.sync.dma_start(t[:], xp_d[:, ka * 512 : kb * 512])
                if i == 0:
                    wqB = wp.tile([128, 896], BF, tag="wqB")
                    nc.sync.dma_start(wqB[:], wq_d[:, 128:1024])
                    wk_sb = wp.tile([128, D_MODEL], BF, tag="wk")
                    nc.sync.dma_start(wk_sb[:], wk_d[:])
                if i == 1:
                    nc.sync.dma_start(consts[:], consts_d[:])
            for n0 in range(1, NT):
                for hf in range(2):
                    t = actp.tile([128, 2048], BF, tag=f"x{n0}_{hf}", name=f"x{n0}_{hf}")
                    xt[(n0, hf)] = t
                    nc.sync.dma_start(
                        t[:], xp_d[:, n0 * 4096 + hf * 2048 : n0 * 4096 + (hf + 1) * 2048]
                    )
                if n0 == 1:
                    wv_sb = wp.tile([128, D_MODEL], BF, tag="wv")
                    nc.sync.dma_start(wv_sb[:], wv_d[:])
                if n0 == 2:
                    wo_sb = wp.tile([128, D_MODEL], BF, tag="wo")
                    nc.sync.dma_start(wo_sb[:], wo_d[:])

            def wq_ap(k):
                if k < 1:
                    return wqA[:]
                return wqB[:, (k - 1) * 128 : k * 128]

            def xap(k, n0):
                if n0 == 0:
                    for i, (ka, kb) in enumerate(x0sl):
                        if ka <= k < kb:
                            return xt[(0, i)][:, (k - ka) * 512 : (k - ka + 1) * 512]
                t = xt[(n0, k // 4)]
                return t[:, (k % 4) * 512 : (k % 4 + 1) * 512]

            # ones columns (64 and 129 of each chunk block) via full-tile fill
            nc.gpsimd.memset(krows[:], 1.0)
            nc.gpsimd.memset(vrows[:], 1.0)

            chains = {}

            def proj(n0, wap, dst, nm):
                p = pp.tile([128, 512], F32, tag="p", name=nm)
                for k in range(KT):
                    nc.tensor.matmul(
                        p[:], wap(k), xap(k, n0), start=(k == 0), stop=(k == KT - 1)
                    )
                rot(dst[:, n0 * 512 : (n0 + 1) * 512], p[:], 512)

            def sq_part(src, n0):
                cs = slice(n0 * 512, (n0 + 1) * 512)
                sq = sqp.tile([128, 512], BF, tag="sq", name="sq")
                nc.vector.tensor_mul(sq[:], src[:, cs], src[:, cs])
                load["v"] += _cost("v", 512, False, 0.5)
                return sq

            def nrm_part(j, n0, sq):
                nrmt = pp.tile([128, 512], F32, tag="p", name=f"nrm{j}")
                nc.tensor.matmul(nrmt[0:2, :], hindt, sq[:], start=True, stop=True)
                nc.vector.tensor_reduce(
                    nrmbuf[:, j * NT + n0 : j * NT + n0 + 1],
                    nrmt[0:2, :],
                    AX.X,
                    AluOpType.max,
                )
                load["v"] += _cost("v", 512)

            def sq_nrm(j, n0, src):
                # per-token norm^2 from the bf16 activations (cheap DVE square)
                nrm_part(j, n0, sq_part(src, n0))

            def trans_pair(src, rows, c0):
                # transpose chunks c0, c0+1 into an adjacent slot pair, then
                # evacuate both with ONE strided copy
                g = tpctr["i"] % 4
                tpctr["i"] += 1
                for u in range(2):
                    nc.tensor.transpose(
                        ktp[:, (2 * g + u) * 128 : (2 * g + u + 1) * 128],
                        src[:, (c0 + u) * CH : (c0 + u + 1) * CH],
                        ident,
                    )
                for u in range(2):
                    rdst = bass.AP(
                        rows[:].tensor,
                        rows[:].offset + (c0 + u) * RST,
                        [[NCH * RST, 128], [65, 2], [1, 64]],
                    )
                    rsrc = bass.AP(
                        ktp[:].tensor,
                        ktp[:].offset + (2 * g + u) * 128,
                        [[1024, 128], [64, 2], [1, 64]],
                    )
                    rot(rdst, rsrc, 128, mult=0.5)

            def chain_su(sp):
                # prefix-state chain: one [65,65] PSUM tile per head
                ca, cb = 2 * sp, 2 * sp + 1
                sus[sp] = []
                for h in range(HPC):
                    if sp == 0:
                        chains[h] = dlp.tile(
                            [65, 65], F32, tag=f"chain{h}", name=f"chain{h}"
                        )
                    chn = chains[h]
                    for cc, st in ((ca, sp == 0), (cb, False)):
                        nc.tensor.matmul(
                            chn[:],
                            krows[:, cc * RST + h * 65 : cc * RST + h * 65 + 65],
                            vrows[:, cc * RST + h * 65 : cc * RST + h * 65 + 65],
                            start=st,
                            stop=(cc == cb),
                            skip_group_check=True,
                        )
                    su = sup.tile([65, 65], BF, tag=f"su{sp}_{h}", name=f"su{sp}_{h}")
                    rot(su[:], chn[:], 65)
                    sus[sp].append(su)

            def scores_mf(sp):
                mark(f"sp{sp}_scores")
                qs = slice(sp * SPAN, (sp + 1) * SPAN)
                cka = slice(sp * SPAN, sp * SPAN + CH)
                ckb = slice(sp * SPAN + CH, (sp + 1) * SPAN)
                for h in range(HPC):
                    hs = slice(h * 64, (h + 1) * 64)
                    ptj = pp.tile([128, 512], F32, tag="p", name="ptj")
                    nc.tensor.matmul(
                        ptj[:, 0:SPAN], kh2[hs, cka], qh2[hs, qs],
                        start=True, stop=True, tile_position=(h * 64, 0),
                    )
                    nc.tensor.matmul(
                        ptj[:, SPAN:384], kh2[hs, ckb], qh2[hs, ckb],
                        start=True, stop=True, tile_position=(h * 64, 0),
                    )
                    # mf = (ptj + a0) * [tri|ones|tri], one fused DVE op
                    mf = mfp.tile([128, 384], BF, tag=f"mf{h}", name=f"mf{h}")
                    nc.vector.scalar_tensor_tensor(
                        mf[:], ptj[:, 0:384], 1.0, maskf, AluOpType.add, AluOpType.mult
                    )
                    load["v"] += _cost("v", 384)
                    mfs[(sp, h)] = mf

            def attn_chunk(sp, cidx, vhr):
                ca, cb = 2 * sp, 2 * sp + 1
                if True:
                    ck = slice(cidx * CH, (cidx + 1) * CH)
                    o512 = pp.tile([128, 512], F32, tag="p", name="o")
                    o = o512[:, 0:130]
                    for h in range(HPC):
                        vra = vrows[:, ca * RST + h * 65 : ca * RST + h * 65 + 65]
                        vrb = vrows[:, cb * RST + h * 65 : cb * RST + h * 65 + 65]
                        mf = mfs[(sp, h)]
                        oh = o[:, h * 65 : (h + 1) * 65]
                        if cidx == ca:
                            parts = ((mf[:, 0:CH], vra),)
                        else:
                            parts = ((mf[:, CH : 2 * CH], vra), (mf[:, 2 * CH :], vrb))
                        for mi, (mm, vv) in enumerate(parts):
                            nc.tensor.matmul(
                                oh, mm, vv,
                                start=(mi == 0),
                                stop=(sp == 0 and mi == len(parts) - 1),
                                skip_group_check=True,
                            )
                        if sp > 0:
                            qsrc = qh2[0:64, ck] if h == 0 else qh2b[0:64, ck]
                            nc.tensor.matmul(
                                oh,
                                qsrc,
                                sus[sp - 1][h][0:64, :],
                                start=False,
                                stop=False,
                                skip_group_check=True,
                            )
                            nc.tensor.matmul(
                                oh,
                                onesrow64,
                                sus[sp - 1][h][64:65, :],
                                start=False,
                                stop=True,
                                tile_position=(64, 0),
                                skip_group_check=True,
                            )
                    # divide: rec = 1/den, vhr = num * rec (both heads at once)
                    rec = recp.tile([128, 2], F32, tag="rec", name="rec")
                    osrc = bass.AP(o512[:].tensor, o512[:].offset + 64, [[512, 128], [65, 2]])
                    nc.vector.reciprocal(rec[:], osrc)
                    load["v"] += _cost("v", 2)
                    half = (cidx - ca) * CH
                    vdst = bass.AP(
                        vhr[:].tensor,
                        vhr[:].offset + half,
                        [[2 * CH, 128], [64, 2], [1, 64]],
                    )
                    vsrc = bass.AP(
                        o512[:].tensor, o512[:].offset, [[512, 128], [65, 2], [1, 64]]
                    )
                    vrec = bass.AP(
                        rec[:].tensor, rec[:].offset, [[2, 128], [1, 2], [0, 64]]
                    )
                    nc.vector.tensor_mul(vdst, vsrc, vrec)
                    load["v"] += _cost("v", 128)

            def attn_span(sp):
                # both heads share one [128,130] PSUM tile per chunk
                mark(f"sp{sp}_attn")
                vhr = vhrp.tile([128, 2 * CH], BF, tag="vhr", name="vhr")
                attn_chunk(sp, 2 * sp, vhr)
                attn_chunk(sp, 2 * sp + 1, vhr)
                return vhr

            def vht_half(sp, vhr, half):
                g = tpctr["i"] % 4
                tpctr["i"] += 1
                sl = 2 * g * 128
                nc.tensor.transpose(
                    ktp[:, sl : sl + 128], vhr[:, half * CH : (half + 1) * CH], ident
                )
                rot(
                    vht[:, (2 * sp + half) * CH : (2 * sp + half + 1) * CH],
                    ktp[:, sl : sl + 128],
                    128,
                    mult=0.5,
                )

            def vht_span(sp, vhr):
                # transpose both chunks into an adjacent slot pair, evacuate
                # with ONE [128,256] copy into vht
                mark(f"sp{sp}_vht")
                g = tpctr["i"] % 4
                tpctr["i"] += 1
                for u in range(2):
                    nc.tensor.transpose(
                        ktp[:, (2 * g + u) * 128 : (2 * g + u + 1) * 128],
                        vhr[:, u * CH : (u + 1) * CH],
                        ident,
                    )
                rot(
                    vht[:, 2 * sp * CH : (2 * sp + 2) * CH],
                    ktp[:, 2 * g * 128 : (2 * g + 2) * 128],
                    256,
                    mult=0.5,
                )

            def outproj_chunk(cidx, ob, half, tail=False):
                ck = slice(cidx * CH, (cidx + 1) * CH)
                for n2 in range(D_MODEL // 512):
                    ns = slice(n2 * 512, (n2 + 1) * 512)
                    opp = pp.tile([128, 512], F32, tag="p", name="opp")
                    nc.tensor.matmul(
                        opp[:], vht[:, ck], wo_sb[:, ns], start=True, stop=True
                    )
                    cl = half * D_MODEL + n2 * 512
                    rot(ob[:, cl : cl + 512], opp[:], 512)
                    if tail:
                        nc.sync.dma_start(out_d[ck, ns], ob[:, cl : cl + 512])
                if not tail:
                    nc.sync.dma_start(
                        out_d[ck, :], ob[:, half * D_MODEL : (half + 1) * D_MODEL]
                    )

            # ====== stage A: q/k projections, norms, k transposes ======
            for n0 in range(NT):
                mark(f"blk{n0}_qk")
                proj(n0, wq_ap, qh2, "pq")
                if n0 > 0:
                    sq_nrm(1, n0 - 1, kh2)
                proj(n0, lambda k: wk_sb[:, k * 128 : (k + 1) * 128], kh2, "pk")
                sq_nrm(0, n0, qh2)
                if n0 > 0:
                    mark(f"blk{n0 - 1}_ktrans")
                    trans_pair(kh2, krows, 4 * (n0 - 1))
                    trans_pair(kh2, krows, 4 * (n0 - 1) + 2)
            mark("blk3_ktrans")
            trans_pair(kh2, krows, 4 * (NT - 1))
            trans_pair(kh2, krows, 4 * (NT - 1) + 2)
            sq_nrm(1, NT - 1, kh2)

            # norm finalize on DVE/Act while the PE streams v projections
            mq = actp.tile([2, 1], F32, tag="mq")
            mk = actp.tile([2, 1], F32, tag="mk")
            nc.vector.tensor_reduce(mq[:], nrmbuf[:, 0:NT], AX.X, AluOpType.max)
            nc.vector.tensor_reduce(mk[:], nrmbuf[:, NT : 2 * NT], AX.X, AluOpType.max)
            prod = actp.tile([2, 1], F32, tag="prod")
            nc.vector.tensor_mul(prod[:], mq[:], mk[:])
            rt = actp.tile([2, 1], F32, tag="rt")
            nc.scalar.activation(rt[:], prod[:], AF.Sqrt)
            rs = actp.tile([2, 1], F32, tag="rs")
            nc.vector.reciprocal(rs[:], rt[:])
            rsb = actp.tile([2, 1], BF, tag="rsb")
            nc.vector.tensor_copy(rsb[:], rs[:])

            # ====== stage A2: v projections + s broadcast + q scale ======
            mark("blk0_v")
            proj(0, lambda k: wv_sb[:, k * 128 : (k + 1) * 128], vcol, "pv")
            scvt = pp.tile([128, 512], F32, tag="p", name="scvt")
            nc.tensor.matmul(scvt[:, 0:1], hindtT, rsb[:], start=True, stop=True)
            nc.vector.tensor_copy(scv128[:], scvt[:, 0:1])
            nc.vector.tensor_scalar_mul(qh2[:], qh2[:], scv128[:])
            nc.vector.tensor_copy(qh2b[:], qh2[64:128, :])
            load["v"] += _cost("v", 2048, False, 0.25)
            for n0 in range(1, NT):
                mark(f"blk{n0}_v")
                proj(n0, lambda k: wv_sb[:, k * 128 : (k + 1) * 128], vcol, "pv")
                if n0 == 2:
                    scores_mf(0)
                if n0 >= 2:
                    mark(f"c{2 * (n0 - 2)}_vtrans0")
                    trans_pair(vcol, vrows, 2 * (n0 - 2))
            chain_su(0)
            if DBG == "qh2":
                nc.sync.dma_start(dbg_d[:, 0:N], qh2[:])
            if DBG == "kh2":
                nc.sync.dma_start(dbg_d[:, 0:N], kh2[:])

            # ====== stage B: v-trans / chain / scores / attn / outproj ======
            for sp in range(NSPAN):
                if sp == 0:
                    mark("blk0_vtrans")
                    trans_pair(vcol, vrows, 4)
                vhr = attn_span(sp)
                if sp < NSPAN - 1:
                    scores_mf(sp + 1)
                if sp > 0:
                    mark(f"sp{sp - 1}_outproj")
                    ob = obp.tile([128, 2 * D_MODEL], BF, tag="ob", name="ob")
                    outproj_chunk(2 * (sp - 1), ob, 0)
                if 0 < sp < NSPAN - 1:
                    chain_su(sp)
                if sp > 0:
                    outproj_chunk(2 * (sp - 1) + 1, ob, 1)
                if sp < NSPAN - 1:
                    vht_span(sp, vhr)
                else:
                    vhr_last = vhr
                if 2 * sp + 6 < NCH:
                    mark(f"c{2 * sp + 6}_vtrans")
                    trans_pair(vcol, vrows, 2 * sp + 6)
            mark(f"sp{NSPAN - 1}_outproj")
            obt = obp.tile([128, 2 * D_MODEL], BF, tag="ob", name="ob")
            vht_half(NSPAN - 1, vhr_last, 0)
            outproj_chunk(2 * (NSPAN - 1), obt, 0, tail=True)
            vht_half(NSPAN - 1, vhr_last, 1)
            outproj_chunk(2 * (NSPAN - 1) + 1, obt, 1, tail=True)
            if DBG == "vht":
                nc.sync.dma_start(dbg_d[:, 0:N], vht[:])
            if DBG == "vrows":
                nc.sync.dma_start(dbg_d[:, 0 : NCH * RST], vrows[:])
            if DBG == "krows":
                nc.sync.dma_start(dbg_d[:, 0 : NCH * RST], krows[:])
            if DBG == "su":
                for spd in range(NSPAN - 1):
                    nc.sync.dma_start(
                        dbg_d[0:65, spd * 65 : (spd + 1) * 65], sus[spd][0][:]
                    )
                    nc.sync.dma_start(
                        dbg_d[0:65, 2000 + spd * 65 : 2000 + (spd + 1) * 65],
                        sus[spd][1][:],
                    )

    nc.compile()
    return nc


def _consts():
    import ml_dtypes

    bf = ml_dtypes.bfloat16
    consts = np.zeros((128, 644), dtype=np.float32)
    consts[:, 0:128] = np.eye(128)
    j = np.arange(128)[:, None]
    i = np.arange(128)[None, :]
    tri = (j <= i).astype(np.float32)
    consts[:, 128:256] = tri
    consts[:, 256:384] = 1.0
    consts[:, 384:512] = tri
    for h in range(HPC):
        consts[h * 64 : (h + 1) * 64, 512 + h] = 1.0
        consts[h, 514 + h * 64 : 514 + (h + 1) * 64] = 1.0
    return consts.astype(bf)


def _in_maps(inputs):
    import ml_dtypes

    bf = ml_dtypes.bfloat16
    X = np.ascontiguousarray(np.asarray(inputs["X"], dtype=np.float32))
    xt = np.ascontiguousarray(X[0].T).astype(bf)  # [D_MODEL, N]
    # pack: [128, NT*KT*512], col n0*4096 + k*512 + c = X^T[k*128+p, n0*512+c]
    xp = np.ascontiguousarray(
        xt.reshape(KT, 128, NT, 512).transpose(1, 2, 0, 3).reshape(128, NT * KT * 512)
    )
    wqt = np.ascontiguousarray(np.asarray(inputs["Wq"], np.float32).T).astype(bf)
    wkt = np.ascontiguousarray(np.asarray(inputs["Wk"], np.float32).T).astype(bf)
    wvt = np.ascontiguousarray(np.asarray(inputs["Wv"], np.float32).T).astype(bf)
    wot = np.ascontiguousarray(np.asarray(inputs["Wo"], np.float32).T).astype(bf)
    consts = _consts()

    def sb_layout(w):  # [1024, 128] -> [128, 8*128] (dm-chunk on partitions)
        return np.ascontiguousarray(
            w.reshape(KT, 128, DPC).transpose(1, 0, 2).reshape(128, KT * DPC)
        )

    in_maps = []
    for c in range(NCORES):
        cs = slice(c * DPC, (c + 1) * DPC)
        in_maps.append(
            {
                "xp": xp,
                "wq": sb_layout(wqt[:, cs]),
                "wk": sb_layout(wkt[:, cs]),
                "wv": sb_layout(wvt[:, cs]),
                "wo": np.ascontiguousarray(wot[cs, :]),
                "consts": consts,
            }
        )
    return in_maps


def _run(inputs, trace=False):
    from concourse.bass_utils import run_bass_kernel_spmd

    if "nc" not in _CACHE:
        _CACHE["nc"] = _build()
    nc = _CACHE["nc"]
    in_maps = _in_maps(inputs)
    res = run_bass_kernel_spmd(nc, in_maps, core_ids=list(range(NCORES)), trace=trace)
    bo = np.asarray(inputs["bo"], dtype=np.float32)
    acc = np.zeros((N, D_MODEL), dtype=np.float32)
    for c in range(NCORES):
        acc += res.results[c]["out"].astype(np.float32)
    acc += bo[None, :]
    return acc.reshape(B, N, D_MODEL), res.exec_time_ns


def kernel(**inputs) -> np.ndarray:
    out, _ = _run(inputs, trace=False)
    return out


# revision 10
# speedup vs baseline: 1.0359x; 1.0093x over previous
"""Fastmax (p=1 causal linear attention) Trainium2 kernel, 8-core SPMD, v5.

Sharding: data-parallel over heads (16 heads -> 2 per core). Each core
computes q/k/v projections for its 2 heads, chunked causal linear attention
(augmented [65,65] prefix state per head), and a partial output projection;
the host sums the 8 partial outputs and adds the bias.

v5 schedule/copy-count rewrite:
  - One shared [128,512]-f32 PSUM ring (4 bufs) serves projections, norm
    rows, scores, attention o tiles and out-proj tiles; transposes run in a
    single [128,512]-bf16 4-slot tile in chunk PAIRS whose results leave in
    ONE strided copy per pair (half the transpose-evacuation ops).
  - Stage A: q/k projections + per-token norms + k transposes (PE-bound);
    the norm finalize runs on DVE/Act under the v projections; q is scaled
    once by s; scores then need no further scaling and a0 is folded into
    mf = (ptj + 1) * [tri|ones|tri] (one fused DVE op per head per span).
  - Stage B interleaves v transposes (two chunks ahead), the prefix-state
    chain, scores, attention, and the one-span-delayed output projection.
  - State snapshots are ONE [65,65] copy per head (base partition 0); the
    head-1 q rows are replicated to partitions 0:63 once (qh2b) so the
    state matmuls' operands share base partitions; the prefix ones-row term
    uses a ones lhsT at partition 64 matching the snapshot row.
"""

import sys

sys.path.insert(0, "/opt/trn_rl_repo")

import numpy as np

B, N, D_MODEL, H, D_HEAD = 1, 2048, 1024, 16, 64
NCORES = 8
HPC = H // NCORES  # heads per core
DPC = HPC * D_HEAD  # out dims per core (128)
CH = 128  # chunk (tokens)
SPAN = 256  # query span (2 chunks)
NSPAN = N // SPAN
NCH = N // CH
KT = D_MODEL // 128  # contraction tiles for projections
NT = N // 512  # 512-wide column chunks of the sequence
RST = 136  # row-buffer stride per chunk (2*(64 data + ones col) + pad)

_CACHE = {}
DBG = None
MARKS = []


def _build():
    import concourse.bass as bass
    import concourse.tile as tile
    import concourse.mybir as mybir
    from concourse import bacc
    from concourse.alu_op_type import AluOpType

    BF = mybir.dt.bfloat16
    F32 = mybir.dt.float32
    AF = mybir.ActivationFunctionType
    AX = mybir.AxisListType

    nc = bacc.Bacc("TRN2", target_bir_lowering=False, debug=False, num_devices=NCORES)

    # xp: host-packed X, col n0*4096 + k*512 + c <-> X[tok n0*512+c, dm k*128+p]
    xp_d = nc.declare_dram_parameter("xp", [128, NT * KT * 512], BF, isOutput=False)
    wq_d = nc.declare_dram_parameter("wq", [128, D_MODEL], BF, isOutput=False)
    wk_d = nc.declare_dram_parameter("wk", [128, D_MODEL], BF, isOutput=False)
    wv_d = nc.declare_dram_parameter("wv", [128, D_MODEL], BF, isOutput=False)
    wo_d = nc.declare_dram_parameter("wo", [DPC, D_MODEL], BF, isOutput=False)
    consts_d = nc.declare_dram_parameter("consts", [128, 644], BF, isOutput=False)
    out_d = nc.declare_dram_parameter("out", [N, D_MODEL], BF, isOutput=True)
    dbg_d = nc.declare_dram_parameter("dbg", [128, 4352], BF, isOutput=True) if DBG else None

    # engine load balancer for PSUM->SBUF copies & small ops (cost-model based)
    load = {"v": 0.0, "s": 0.0}

    def _cost(eng, n, psum_src=True, mult=1.0):
        if eng == "v":
            return n * 1.04 * mult + (125.0 if psum_src else 60.0)
        return n * 0.92 + (160.0 if psum_src else 200.0)

    def mark(label):
        MARKS.append((label, int(nc.get_next_instruction_name()[2:])))

    with tile.TileContext(nc) as tc:

        def pick(n, psum_src=True, mult=1.0):
            cv = load["v"] + _cost("v", n, psum_src, mult)
            cs = load["s"] + _cost("s", n, psum_src)
            if cv <= cs:
                load["v"] = cv
                return "v"
            load["s"] = cs
            return "s"

        def rot(dst, src, n, psum_src=True, mult=1.0):
            eng = pick(n, psum_src, mult)
            if eng == "v":
                nc.vector.tensor_copy(dst, src)
            else:
                nc.scalar.copy(dst, src)

        with (
            tc.tile_pool(name="const", bufs=1) as constp,
            tc.tile_pool(name="wqkv", bufs=1) as wp,
            tc.tile_pool(name="acts", bufs=1) as actp,
            tc.tile_pool(name="mfp", bufs=2) as mfp,
            tc.tile_pool(name="sup", bufs=1) as sup,
            tc.tile_pool(name="vhrp", bufs=4) as vhrp,
            tc.tile_pool(name="obp", bufs=3) as obp,
            tc.tile_pool(name="sqp", bufs=3) as sqp,
            tc.tile_pool(name="recp", bufs=6) as recp,
            tc.tile_pool(name="ktpp", bufs=1, space="PSUM") as ktpp,
            tc.tile_pool(name="dlp", bufs=1, space="PSUM") as dlp,
            tc.tile_pool(name="pp", bufs=5, space="PSUM") as pp,
        ):
            consts = constp.tile([128, 644], BF)
            ident = consts[:, 0:128]
            maskf = consts[:, 128:512]  # [tri 128 | ones 128 | tri 128]
            onesrow64 = consts[64:65, 256:384]  # ones row [1,128] at partition 64
            hindt = consts[:, 512:514]  # per-head indicator [128,2]
            hindtT = consts[0:2, 514:642]  # transposed indicator [2,128]

            # warm up Act function tables off the critical path
            warm = actp.tile([1, 1], F32, tag="warm")
            nc.gpsimd.memset(warm[:], 1.0)
            warm2 = actp.tile([1, 1], F32, tag="warm2")
            nc.scalar.activation(warm2[:], warm[:], AF.Sqrt)

            # persistent activations
            qh2 = actp.tile([128, N], BF, tag="qh2")
            qh2b = actp.tile([64, N], BF, tag="qh2b")  # head-1 q at partitions 0:63
            kh2 = actp.tile([128, N], BF, tag="kh2")
            vcol = actp.tile([128, N], BF, tag="vcol")
            vht = actp.tile([128, N], BF, tag="vht")
            krows = actp.tile([128, NCH * RST], BF, tag="krows")
            vrows = actp.tile([128, NCH * RST], BF, tag="vrows")
            nrmbuf = actp.tile([2, 2 * NT], F32, tag="nrmbuf")
            scv128 = actp.tile([128, 1], F32, tag="scv128")
            sus = {}
            mfs = {}

            # transposes run in pairs through a 4-slot bf16 PSUM tile
            ktp = ktpp.tile([128, 1024], BF, tag="tp", name="tp")
            tpctr = {"i": 0}

            # weights and X tiles, issued in first-use order; block-0 operands
            # ship in small pieces so the PE starts as early as possible
            wqA = wp.tile([128, 128], BF, tag="wqA")
            nc.sync.dma_start(wqA[:], wq_d[:, 0:128])
            xt = {}
            x0sl = [(0, 4), (4, 8)]
            for i, (ka, kb) in enumerate(x0sl):
                t = actp.tile([128, (kb - ka) * 512], BF, tag=f"x0_{i}", name=f"x0_{i}")
                xt[(0, i)] = t
                nc.sync.dma_start(t[:], xp_d[:, ka * 512 : kb * 512])
                if i == 0:
                    wqB = wp.tile([128, 896], BF, tag="wqB")
                    nc.sync.dma_start(wqB[:], wq_d[:, 128:1024])
                    wk_sb = wp.tile([128, D_MODEL], BF, tag="wk")
                    nc.sync.dma_start(wk_sb[:], wk_d[:])
                if i == 1:
                    nc.sync.dma_start(consts[:], consts_d[:])
            for n0 in range(1, NT):
                for hf in range(2):
                    t = actp.tile([128, 2048], BF, tag=f"x{n0}_{hf}", name=f"x{n0}_{hf}")
                    xt[(n0, hf)] = t
                    nc.sync.dma_start(
                        t[:], xp_d[:, n0 * 4096 + hf * 2048 : n0 * 4096 + (hf + 1) * 2048]
                    )
                if n0 == 1:
                    wv_sb = wp.tile([128, D_MODEL], BF, tag="wv")
                    nc.sync.dma_start(wv_sb[:], wv_d[:])
                if n0 == 2:
                    wo_sb = wp.tile([128, D_MODEL], BF, tag="wo")
                    nc.sync.dma_start(wo_sb[:], wo_d[:])

            def wq_ap(k):
                if k < 1:
                    return wqA[:]
                return wqB[:, (k - 1) * 128 : k * 128]

            def xap(k, n0):
                if n0 == 0:
                    for i, (ka, kb) in enumerate(x0sl):
                        if ka <= k < kb:
                            return xt[(0, i)][:, (k - ka) * 512 : (k - ka + 1) * 512]
                t = xt[(n0, k // 4)]
                return t[:, (k % 4) * 512 : (k % 4 + 1) * 512]

            # ones columns (64 and 129 of each chunk block) via full-tile fill
            nc.gpsimd.memset(krows[:], 1.0)
            nc.gpsimd.memset(vrows[:], 1.0)

            chains = {}

            def proj(n0, wap, dst, nm):
                p = pp.tile([128, 512], F32, tag="p", name=nm)
                for k in range(KT):
                    nc.tensor.matmul(
                        p[:], wap(k), xap(k, n0), start=(k == 0), stop=(k == KT - 1)
                    )
                rot(dst[:, n0 * 512 : (n0 + 1) * 512], p[:], 512)

            def sq_part(src, n0):
                cs = slice(n0 * 512, (n0 + 1) * 512)
                sq = sqp.tile([128, 512], BF, tag="sq", name="sq")
                nc.vector.tensor_mul(sq[:], src[:, cs], src[:, cs])
                load["v"] += _cost("v", 512, False, 0.5)
                return sq

            def nrm_part(j, n0, sq):
                nrmt = pp.tile([128, 512], F32, tag="p", name=f"nrm{j}")
                nc.tensor.matmul(nrmt[0:2, :], hindt, sq[:], start=True, stop=True)
                nc.vector.tensor_reduce(
                    nrmbuf[:, j * NT + n0 : j * NT + n0 + 1],
                    nrmt[0:2, :],
                    AX.X,
                    AluOpType.max,
                )
                load["v"] += _cost("v", 512)

            def sq_nrm(j, n0, src):
                # per-token norm^2 from the bf16 activations (cheap DVE square)
                nrm_part(j, n0, sq_part(src, n0))

            def trans_pair(src, rows, c0):
                # transpose chunks c0, c0+1 into an adjacent slot pair, then
                # evacuate both with ONE strided copy
                g = tpctr["i"] % 4
                tpctr["i"] += 1
                for u in range(2):
                    nc.tensor.transpose(
                        ktp[:, (2 * g + u) * 128 : (2 * g + u + 1) * 128],
                        src[:, (c0 + u) * CH : (c0 + u + 1) * CH],
                        ident,
                    )
                for u in range(2):
                    rdst = bass.AP(
                        rows[:].tensor,
                        rows[:].offset + (c0 + u) * RST,
                        [[NCH * RST, 128], [65, 2], [1, 64]],
                    )
                    rsrc = bass.AP(
                        ktp[:].tensor,
                        ktp[:].offset + (2 * g + u) * 128,
                        [[1024, 128], [64, 2], [1, 64]],
                    )
                    rot(rdst, rsrc, 128, mult=0.5)

            def chain_su(sp):
                # prefix-state chain: one [65,65] PSUM tile per head
                ca, cb = 2 * sp, 2 * sp + 1
                sus[sp] = []
                for h in range(HPC):
                    if sp == 0:
                        chains[h] = dlp.tile(
                            [65, 65], F32, tag=f"chain{h}", name=f"chain{h}"
                        )
                    chn = chains[h]
                    for cc, st in ((ca, sp == 0), (cb, False)):
                        nc.tensor.matmul(
                            chn[:],
                            krows[:, cc * RST + h * 65 : cc * RST + h * 65 + 65],
                            vrows[:, cc * RST + h * 65 : cc * RST + h * 65 + 65],
                            start=st,
                            stop=(cc == cb),
                            skip_group_check=True,
                        )
                    su = sup.tile([65, 65], BF, tag=f"su{sp}_{h}", name=f"su{sp}_{h}")
                    rot(su[:], chn[:], 65)
                    sus[sp].append(su)

            def scores_mf(sp):
                mark(f"sp{sp}_scores")
                qs = slice(sp * SPAN, (sp + 1) * SPAN)
                cka = slice(sp * SPAN, sp * SPAN + CH)
                ckb = slice(sp * SPAN + CH, (sp + 1) * SPAN)
                for h in range(HPC):
                    hs = slice(h * 64, (h + 1) * 64)
                    ptj = pp.tile([128, 512], F32, tag="p", name="ptj")
                    nc.tensor.matmul(
                        ptj[:, 0:SPAN], kh2[hs, cka], qh2[hs, qs],
                        start=True, stop=True, tile_position=(h * 64, 0),
                    )
                    nc.tensor.matmul(
                        ptj[:, SPAN:384], kh2[hs, ckb], qh2[hs, ckb],
                        start=True, stop=True, tile_position=(h * 64, 0),
                    )
                    # mf = (ptj + a0) * [tri|ones|tri], one fused DVE op
                    mf = mfp.tile([128, 384], BF, tag=f"mf{h}", name=f"mf{h}")
                    nc.vector.scalar_tensor_tensor(
                        mf[:], ptj[:, 0:384], 1.0, maskf, AluOpType.add, AluOpType.mult
                    )
                    load["v"] += _cost("v", 384)
                    mfs[(sp, h)] = mf

            def attn_chunk(sp, cidx, vhr):
                ca, cb = 2 * sp, 2 * sp + 1
                if True:
                    ck = slice(cidx * CH, (cidx + 1) * CH)
                    o512 = pp.tile([128, 512], F32, tag="p", name="o")
                    o = o512[:, 0:130]
                    for h in range(HPC):
                        vra = vrows[:, ca * RST + h * 65 : ca * RST + h * 65 + 65]
                        vrb = vrows[:, cb * RST + h * 65 : cb * RST + h * 65 + 65]
                        mf = mfs[(sp, h)]
                        oh = o[:, h * 65 : (h + 1) * 65]
                        if cidx == ca:
                            parts = ((mf[:, 0:CH], vra),)
                        else:
                            parts = ((mf[:, CH : 2 * CH], vra), (mf[:, 2 * CH :], vrb))
                        for mi, (mm, vv) in enumerate(parts):
                            nc.tensor.matmul(
                                oh, mm, vv,
                                start=(mi == 0),
                                stop=(sp == 0 and mi == len(parts) - 1),
                                skip_group_check=True,
                            )
                        if sp > 0:
                            qsrc = qh2[0:64, ck] if h == 0 else qh2b[0:64, ck]
                            nc.tensor.matmul(
                                oh,
                                qsrc,
                                sus[sp - 1][h][0:64, :],
                                start=False,
                                stop=False,
                                skip_group_check=True,
                            )
                            nc.tensor.matmul(
                                oh,
                                onesrow64,
                                sus[sp - 1][h][64:65, :],
                                start=False,
                                stop=True,
                                tile_position=(64, 0),
                                skip_group_check=True,
                            )
                    # divide: rec = 1/den, vhr = num * rec (both heads at once)
                    rec = recp.tile([128, 2], F32, tag="rec", name="rec")
                    osrc = bass.AP(o512[:].tensor, o512[:].offset + 64, [[512, 128], [65, 2]])
                    nc.vector.reciprocal(rec[:], osrc)
                    load["v"] += _cost("v", 2)
                    half = (cidx - ca) * CH
                    vdst = bass.AP(
                        vhr[:].tensor,
                        vhr[:].offset + half,
                        [[2 * CH, 128], [64, 2], [1, 64]],
                    )
                    vsrc = bass.AP(
                        o512[:].tensor, o512[:].offset, [[512, 128], [65, 2], [1, 64]]
                    )
                    vrec = bass.AP(
                        rec[:].tensor, rec[:].offset, [[2, 128], [1, 2], [0, 64]]
                    )
                    nc.vector.tensor_mul(vdst, vsrc, vrec)
                    load["v"] += _cost("v", 128)

            def attn_span(sp):
                # both heads share one [128,130] PSUM tile per chunk
                mark(f"sp{sp}_attn")
                vhr = vhrp.tile([128, 2 * CH], BF, tag="vhr", name="vhr")
                attn_chunk(sp, 2 * sp, vhr)
                attn_chunk(sp, 2 * sp + 1, vhr)
                return vhr

            def vht_half(sp, vhr, half):
                g = tpctr["i"] % 4
                tpctr["i"] += 1
                sl = 2 * g * 128
                nc.tensor.transpose(
                    ktp[:, sl : sl + 128], vhr[:, half * CH : (half + 1) * CH], ident
                )
                rot(
                    vht[:, (2 * sp + half) * CH : (2 * sp + half + 1) * CH],
                    ktp[:, sl : sl + 128],
                    128,
                    mult=0.5,
                )

            def vht_span(sp, vhr):
                # transpose both chunks into an adjacent slot pair, evacuate
                # with ONE [128,256] copy into vht
                mark(f"sp{sp}_vht")
                g = tpctr["i"] % 4
                tpctr["i"] += 1
                for u in range(2):
                    nc.tensor.transpose(
                        ktp[:, (2 * g + u) * 128 : (2 * g + u + 1) * 128],
                        vhr[:, u * CH : (u + 1) * CH],
                        ident,
                    )
                rot(
                    vht[:, 2 * sp * CH : (2 * sp + 2) * CH],
                    ktp[:, 2 * g * 128 : (2 * g + 2) * 128],
                    256,
                    mult=0.5,
                )

            def outproj_chunk(cidx, ob, half, tail=False):
                ck = slice(cidx * CH, (cidx + 1) * CH)
                for n2 in range(D_MODEL // 512):
                    ns = slice(n2 * 512, (n2 + 1) * 512)
                    opp = pp.tile([128, 512], F32, tag="p", name="opp")
                    nc.tensor.matmul(
                        opp[:], vht[:, ck], wo_sb[:, ns], start=True, stop=True
                    )
                    cl = half * D_MODEL + n2 * 512
                    rot(ob[:, cl : cl + 512], opp[:], 512)
                    if tail:
                        nc.sync.dma_start(out_d[ck, ns], ob[:, cl : cl + 512])
                if not tail:
                    nc.sync.dma_start(
                        out_d[ck, :], ob[:, half * D_MODEL : (half + 1) * D_MODEL]
                    )

            # ====== stage A: q/k projections, norms, k transposes ======
            for n0 in range(NT):
                mark(f"blk{n0}_qk")
                proj(n0, wq_ap, qh2, "pq")
                if n0 > 0:
                    sq_nrm(1, n0 - 1, kh2)
                proj(n0, lambda k: wk_sb[:, k * 128 : (k + 1) * 128], kh2, "pk")
                sq_nrm(0, n0, qh2)
                if n0 > 0:
                    mark(f"blk{n0 - 1}_ktrans")
                    trans_pair(kh2, krows, 4 * (n0 - 1))
                    trans_pair(kh2, krows, 4 * (n0 - 1) + 2)
            mark("blk3_ktrans")
            trans_pair(kh2, krows, 4 * (NT - 1))
            trans_pair(kh2, krows, 4 * (NT - 1) + 2)
            sq_nrm(1, NT - 1, kh2)

            mark("blk3_ktrans")
            trans_pair(kh2, krows, 4 * (NT - 1))
            trans_pair(kh2, krows, 4 * (NT - 1) + 2)

            # norm finalize on DVE/Act while the PE streams v projections
            mq = actp.tile([2, 1], F32, tag="mq")
            mk = actp.tile([2, 1], F32, tag="mk")
            nc.vector.tensor_reduce(mq[:], nrmbuf[:, 0:NT], AX.X, AluOpType.max)
            nc.vector.tensor_reduce(mk[:], nrmbuf[:, NT : 2 * NT], AX.X, AluOpType.max)
            prod = actp.tile([2, 1], F32, tag="prod")
            nc.vector.tensor_mul(prod[:], mq[:], mk[:])
            rt = actp.tile([2, 1], F32, tag="rt")
            nc.scalar.activation(rt[:], prod[:], AF.Sqrt)
            rs = actp.tile([2, 1], F32, tag="rs")
            nc.vector.reciprocal(rs[:], rt[:])
            rsb = actp.tile([2, 1], BF, tag="rsb")
            nc.vector.tensor_copy(rsb[:], rs[:])

            # ====== stage A2: v projections + s broadcast + q scale ======
            mark("blk0_v")
            proj(0, lambda k: wv_sb[:, k * 128 : (k + 1) * 128], vcol, "pv")
            scvt = pp.tile([128, 512], F32, tag="p", name="scvt")
            nc.tensor.matmul(scvt[:, 0:1], hindtT, rsb[:], start=True, stop=True)
            nc.vector.tensor_copy(scv128[:], scvt[:, 0:1])
            nc.vector.tensor_scalar_mul(qh2[:], qh2[:], scv128[:])
            nc.vector.tensor_copy(qh2b[:], qh2[64:128, :])
            load["v"] += _cost("v", 2048, False, 0.25)
            for n0 in range(1, NT):
                mark(f"blk{n0}_v")
                proj(n0, lambda k: wv_sb[:, k * 128 : (k + 1) * 128], vcol, "pv")
                if n0 == 2:
                    scores_mf(0)
                if n0 >= 2:
                    mark(f"c{2 * (n0 - 2)}_vtrans0")
                    trans_pair(vcol, vrows, 2 * (n0 - 2))
            chain_su(0)
            if DBG == "qh2":
                nc.sync.dma_start(dbg_d[:, 0:N], qh2[:])
            if DBG == "kh2":
                nc.sync.dma_start(dbg_d[:, 0:N], kh2[:])

            # ====== stage B: v-trans / chain / scores / attn / outproj ======
            for sp in range(NSPAN):
                if sp == 0:
                    mark("blk0_vtrans")
                    trans_pair(vcol, vrows, 4)
                vhr = attn_span(sp)
                if sp < NSPAN - 1:
                    scores_mf(sp + 1)
                if sp > 0:
                    mark(f"sp{sp - 1}_outproj")
                    ob = obp.tile([128, 2 * D_MODEL], BF, tag="ob", name="ob")
                    outproj_chunk(2 * (sp - 1), ob, 0)
                if 0 < sp < NSPAN - 1:
                    chain_su(sp)
                if sp > 0:
                    outproj_chunk(2 * (sp - 1) + 1, ob, 1)
                if sp < NSPAN - 1:
                    vht_span(sp, vhr)
                else:
                    vhr_last = vhr
                if 2 * sp + 6 < NCH:
                    mark(f"c{2 * sp + 6}_vtrans")
                    trans_pair(vcol, vrows, 2 * sp + 6)
            mark(f"sp{NSPAN - 1}_outproj")
            obt = obp.tile([128, 2 * D_MODEL], BF, tag="ob", name="ob")
            vht_half(NSPAN - 1, vhr_last, 0)
            outproj_chunk(2 * (NSPAN - 1), obt, 0, tail=True)
            vht_half(NSPAN - 1, vhr_last, 1)
            outproj_chunk(2 * (NSPAN - 1) + 1, obt, 1, tail=True)
            if DBG == "vht":
                nc.sync.dma_start(dbg_d[:, 0:N], vht[:])
            if DBG == "vrows":
                nc.sync.dma_start(dbg_d[:, 0 : NCH * RST], vrows[:])
            if DBG == "krows":
                nc.sync.dma_start(dbg_d[:, 0 : NCH * RST], krows[:])
            if DBG == "su":
                for spd in range(NSPAN - 1):
                    nc.sync.dma_start(
                        dbg_d[0:65, spd * 65 : (spd + 1) * 65], sus[spd][0][:]
                    )
                    nc.sync.dma_start(
                        dbg_d[0:65, 2000 + spd * 65 : 2000 + (spd + 1) * 65],
                        sus[spd][1][:],
                    )

    nc.compile()
    return nc


def _consts():
    import ml_dtypes

    bf = ml_dtypes.bfloat16
    consts = np.zeros((128, 644), dtype=np.float32)
    consts[:, 0:128] = np.eye(128)
    j = np.arange(128)[:, None]
    i = np.arange(128)[None, :]
    tri = (j <= i).astype(np.float32)
    consts[:, 128:256] = tri
    consts[:, 256:384] = 1.0
    consts[:, 384:512] = tri
    for h in range(HPC):
        consts[h * 64 : (h + 1) * 64, 512 + h] = 1.0
        consts[h, 514 + h * 64 : 514 + (h + 1) * 64] = 1.0
    return consts.astype(bf)


def _in_maps(inputs):
    import ml_dtypes

    bf = ml_dtypes.bfloat16
    X = np.ascontiguousarray(np.asarray(inputs["X"], dtype=np.float32))
    xt = np.ascontiguousarray(X[0].T).astype(bf)  # [D_MODEL, N]
    # pack: [128, NT*KT*512], col n0*4096 + k*512 + c = X^T[k*128+p, n0*512+c]
    xp = np.ascontiguousarray(
        xt.reshape(KT, 128, NT, 512).transpose(1, 2, 0, 3).reshape(128, NT * KT * 512)
    )
    wqt = np.ascontiguousarray(np.asarray(inputs["Wq"], np.float32).T).astype(bf)
    wkt = np.ascontiguousarray(np.asarray(inputs["Wk"], np.float32).T).astype(bf)
    wvt = np.ascontiguousarray(np.asarray(inputs["Wv"], np.float32).T).astype(bf)
    wot = np.ascontiguousarray(np.asarray(inputs["Wo"], np.float32).T).astype(bf)
    consts = _consts()

    def sb_layout(w):  # [1024, 128] -> [128, 8*128] (dm-chunk on partitions)
        return np.ascontiguousarray(
            w.reshape(KT, 128, DPC).transpose(1, 0, 2).reshape(128, KT * DPC)
        )

    in_maps = []
    for c in range(NCORES):
        cs = slice(c * DPC, (c + 1) * DPC)
        in_maps.append(
            {
                "xp": xp,
                "wq": sb_layout(wqt[:, cs]),
                "wk": sb_layout(wkt[:, cs]),
                "wv": sb_layout(wvt[:, cs]),
                "wo": np.ascontiguousarray(wot[cs, :]),
                "consts": consts,
            }
        )
    return in_maps


def _run(inputs, trace=False):
    from concourse.bass_utils import run_bass_kernel_spmd

    if "nc" not in _CACHE:
        _CACHE["nc"] = _build()
    nc = _CACHE["nc"]
    in_maps = _in_maps(inputs)
    res = run_bass_kernel_spmd(nc, in_maps, core_ids=list(range(NCORES)), trace=trace)
    bo = np.asarray(inputs["bo"], dtype=np.float32)
    acc = np.zeros((N, D_MODEL), dtype=np.float32)
    for c in range(NCORES):
        acc += res.results[c]["out"].astype(np.float32)
    acc += bo[None, :]
    return acc.reshape(B, N, D_MODEL), res.exec_time_ns


def kernel(**inputs) -> np.ndarray:
    out, _ = _run(inputs, trace=False)
    return out
